# revision 1
# baseline (speedup 1.0000x reference)
"""Trainium2 Bass kernel for nn_DecoderLayer_19816979104174.

Data-parallel over batch: each of the 8 NeuronCores runs one batch element's
full decoder layer. All matmuls in bf16 (fp32 PSUM accumulation). Attention is
computed in transposed [s, t] layout so that:
  - Q/K/V projections consume a single on-chip transpose of x,
  - softmax row-sums come from ones-column matmuls on the PE,
  - the attention-weighted sums feed the output projection with no transposes.
Causal structure is exploited by never computing s>t blocks (the exp buffer is
zero-initialized once; zeros persist across heads). The output projection is
accumulated per-head into an SBUF fp32 accumulator to bound SBUF usage.
"""

import sys

sys.path.insert(0, "/opt/trn_rl_repo")
sys.path.insert(0, "/root/.axon_site/_ro/trn_rl_repo")

import numpy as np

B, T, S, D, H, F = 8, 1024, 1024, 512, 8, 2048
P = 128
NT, ND, NS, NF = T // P, D // P, S // P, F // P
NC2 = T // 512  # 512-wide t chunks
SCALE = 1.0 / float(np.sqrt(D))
LN_EPS = 1e-5

_CACHE = {}


def _build():
    if "nc" in _CACHE:
        return _CACHE["nc"]

    import concourse.tile as tile
    import concourse.mybir as mybir
    from concourse import bacc
    from concourse.masks import make_identity
    from contextlib import ExitStack

    bf16 = mybir.dt.bfloat16
    f32 = mybir.dt.float32
    AF = mybir.ActivationFunctionType
    OP = mybir.AluOpType

    nc = bacc.Bacc("TRN2")

    # ---- DRAM I/O -----------------------------------------------------
    d_x = nc.dram_tensor("x32", [T, D], f32, kind="ExternalInput")
    d_wq = nc.dram_tensor("wq", [H, D, D], bf16, kind="ExternalInput")
    d_wk = nc.dram_tensor("wk", [H, D, D], bf16, kind="ExternalInput")
    d_wv = nc.dram_tensor("wv", [H, D, D], bf16, kind="ExternalInput")
    d_wqm = nc.dram_tensor("wqm", [H, D, D], bf16, kind="ExternalInput")
    d_wo = nc.dram_tensor("wo", [H * D, D], bf16, kind="ExternalInput")
    d_wom = nc.dram_tensor("wom", [H * D, D], bf16, kind="ExternalInput")
    d_w1 = nc.dram_tensor("w1", [D, F], bf16, kind="ExternalInput")
    d_w2 = nc.dram_tensor("w2", [F, D], bf16, kind="ExternalInput")
    d_bq = nc.dram_tensor("bq_c", [P, H * ND], f32, kind="ExternalInput")
    d_bk = nc.dram_tensor("bk_c", [P, H * ND], f32, kind="ExternalInput")
    d_bqm = nc.dram_tensor("bqm_c", [P, H * ND], f32, kind="ExternalInput")
    d_b1 = nc.dram_tensor("b1_c", [P, NF], f32, kind="ExternalInput")
    d_bo = nc.dram_tensor("bo_row", [1, D], bf16, kind="ExternalInput")
    d_bom = nc.dram_tensor("bom_row", [1, D], bf16, kind="ExternalInput")
    d_b2 = nc.dram_tensor("b2_row", [1, D], bf16, kind="ExternalInput")
    d_memk = nc.dram_tensor("memk", [S, D], bf16, kind="ExternalInput")
    d_memv = nc.dram_tensor("memv", [S, D], bf16, kind="ExternalInput")
    d_tpad = nc.dram_tensor("tpad", [P, NS], f32, kind="ExternalInput")
    d_spad = nc.dram_tensor("spad", [P, NS], f32, kind="ExternalInput")
    d_diag = nc.dram_tensor("diag", [P, P], f32, kind="ExternalInput")
    d_out = nc.dram_tensor("out", [T, D], f32, kind="ExternalOutput")

    with tile.TileContext(nc) as tc, ExitStack() as ctx:
        const = ctx.enter_context(tc.tile_pool(name="const", bufs=1))
        small = ctx.enter_context(tc.tile_pool(name="small", bufs=2))
        psum_mm = ctx.enter_context(tc.tile_pool(name="psum_mm", bufs=4, space="PSUM"))
        psum_tr = ctx.enter_context(tc.tile_pool(name="psum_tr", bufs=2, space="PSUM"))
        psum_rs = ctx.enter_context(tc.tile_pool(name="psum_rs", bufs=2, space="PSUM"))

        # ---- constants / small inputs --------------------------------
        ident_b = const.tile([P, P], bf16)
        make_identity(nc, ident_b)
        ident_f = const.tile([P, P], f32)
        make_identity(nc, ident_f)
        ones_col = const.tile([P, 1], bf16)
        nc.vector.memset(ones_col[:], 1.0)
        ones_row = const.tile([1, P], bf16)
        nc.vector.memset(ones_row[:], 1.0)
        eps_t = const.tile([P, 1], f32)
        nc.vector.memset(eps_t[:], LN_EPS)
        diag_sb = const.tile([P, P], f32)
        nc.gpsimd.dma_start(out=diag_sb[:], in_=d_diag.ap())
        tpad_sb = const.tile([P, NS], f32)
        nc.gpsimd.dma_start(out=tpad_sb[:], in_=d_tpad.ap())
        spad_sb = const.tile([P, NS], f32)
        nc.gpsimd.dma_start(out=spad_sb[:], in_=d_spad.ap())
        bq_sb = const.tile([P, H * ND], f32)
        nc.gpsimd.dma_start(out=bq_sb[:], in_=d_bq.ap())
        bk_sb = const.tile([P, H * ND], f32)
        nc.gpsimd.dma_start(out=bk_sb[:], in_=d_bk.ap())
        bqm_sb = const.tile([P, H * ND], f32)
        nc.gpsimd.dma_start(out=bqm_sb[:], in_=d_bqm.ap())
        b1_sb = const.tile([P, NF], f32)
        nc.gpsimd.dma_start(out=b1_sb[:], in_=d_b1.ap())
        bo_sb = const.tile([1, D], bf16)
        nc.gpsimd.dma_start(out=bo_sb[:], in_=d_bo.ap())
        bom_sb = const.tile([1, D], bf16)
        nc.gpsimd.dma_start(out=bom_sb[:], in_=d_bom.ap())
        b2_sb = const.tile([1, D], bf16)
        nc.gpsimd.dma_start(out=b2_sb[:], in_=d_b2.ap())

        # ---- pools with phase-scoped lifetimes (LIFO close order) ----
        es_x2 = ExitStack()     # x2/x2T: phases 4-5
        x2_pool = es_x2.enter_context(tc.tile_pool(name="x2p", bufs=1))
        es_attn = ExitStack()   # expT + acc: phases 1-4
        attn_pool = es_attn.enter_context(tc.tile_pool(name="attn", bufs=1))
        es_x1 = ExitStack()     # x1/x1T: phases 2-4
        x1_pool = es_x1.enter_context(tc.tile_pool(name="x1p", bufs=1))
        es_x32 = ExitStack()    # x resident: phases 0-2
        x32_pool = es_x32.enter_context(tc.tile_pool(name="x32p", bufs=1))

        expT = attn_pool.tile([P, NS, T], bf16, tag="expT")
        nc.gpsimd.memset(expT[:], 0.0)
        acc_sb = attn_pool.tile([P, NT, D], f32, tag="acc")
        x32_sb = x32_pool.tile([P, NT, D], f32)
        for tb in range(NT):
            nc.sync.dma_start(
                out=x32_sb[:, tb, :],
                in_=d_x.ap().rearrange("(tb p) d -> p tb d", p=P)[:, tb, :])

        def transpose_to(src_ap, dstT, ident, dt_blocks, tb, dtype_ps):
            for dt in range(dt_blocks):
                tr_ps = psum_tr.tile([P, P], dtype_ps, tag="tr")
                nc.tensor.transpose(
                    tr_ps[:], src_ap[:, dt * P:(dt + 1) * P], ident[:])
                nc.vector.tensor_copy(dstT[:, dt, tb * P:(tb + 1) * P], tr_ps[:])

        def layernorm(src_ap, resid_ap, dst_ap):
            res = small.tile([P, D], f32, tag="ln_res")
            nc.vector.tensor_tensor(out=res[:], in0=src_ap, in1=resid_ap, op=OP.add)
            stats = small.tile([P, 6], f32, tag="ln_stats")
            nc.vector.bn_stats(stats[:], res[:])
            mv = small.tile([P, 2], f32, tag="ln_mv")
            nc.vector.bn_aggr(mv[:], stats[:])
            std = small.tile([P, 1], f32, tag="ln_std")
            nc.scalar.activation(std[:], mv[:, 1:2], AF.Sqrt, bias=eps_t[:])
            istd = small.tile([P, 1], f32, tag="ln_istd")
            nc.vector.reciprocal(istd[:], std[:])
            nc.vector.tensor_scalar(
                out=dst_ap, in0=res[:], scalar1=mv[:, 0:1], scalar2=istd[:],
                op0=OP.subtract, op1=OP.mult)

        def attention(qT, kT_ap, v_ap, pad_sb, causal, rbc_pool, hoT_pool):
            """softmax(scale * kT.T-x-qT + pad) -> hoT [e, t]; returns hoT."""
            recipT = rbc_pool.tile([1, T], f32, tag="recipT")
            recip_bc = rbc_pool.tile([P, T], f32, tag="recip_bc")

            def rowsum_chunk(c):
                # emit as soon as the last s-block feeding chunk c is exp'd,
                # so the recip/broadcast chain hides behind later PE work
                jmax = min(4 * (c + 1), NS) if causal else NS
                rs_ps = psum_rs.tile([1, 512], f32, tag="rs")
                for j in range(jmax):
                    nc.tensor.matmul(
                        rs_ps[:], lhsT=ones_col[:],
                        rhs=expT[:, j, c * 512:(c + 1) * 512],
                        start=(j == 0), stop=(j == jmax - 1))
                sl = slice(c * 512, (c + 1) * 512)
                nc.vector.reciprocal(recipT[:, sl], rs_ps[:])
                nc.gpsimd.partition_broadcast(recip_bc[:, sl], recipT[:, sl])

            for j in range(NS):
                c_lo = (j * P) // 512 if causal else 0
                for c in range(c_lo, NC2):
                    lo = max(j * P, c * 512) if causal else c * 512
                    w = (c + 1) * 512 - lo
                    att_ps = psum_mm.tile([P, 512], f32, tag="mm")
                    for et in range(ND):
                        nc.tensor.matmul(
                            att_ps[:, :w],
                            lhsT=kT_ap[:, et, j * P:(j + 1) * P],
                            rhs=qT[:, et, lo:(c + 1) * 512],
                            start=(et == 0), stop=(et == ND - 1))
                    if causal and lo == j * P:
                        nc.vector.tensor_tensor(
                            out=att_ps[:, 0:P], in0=att_ps[:, 0:P],
                            in1=diag_sb[:], op=OP.add)
                    nc.scalar.activation(
                        expT[:, j, lo:(c + 1) * 512], att_ps[:, :w], AF.Exp,
                        bias=pad_sb[:, j:j + 1], scale=SCALE)
                if causal and j == 3:
                    rowsum_chunk(0)
            if causal:
                rowsum_chunk(1)
            else:
                rowsum_chunk(0)
                rowsum_chunk(1)
            hoT = hoT_pool.tile([P, ND, T], bf16, tag="hoT")
            for eb in range(ND):
                for c in range(NC2):
                    jmax = min(4 * (c + 1), NS) if causal else NS
                    ho_ps = psum_mm.tile([P, 512], f32, tag="mm")
                    for j in range(jmax):
                        nc.tensor.matmul(
                            ho_ps[:],
                            lhsT=v_ap[:, j, eb * P:(eb + 1) * P],
                            rhs=expT[:, j, c * 512:(c + 1) * 512],
                            start=(j == 0), stop=(j == jmax - 1))
                    nc.vector.tensor_tensor(
                        out=hoT[:, eb, c * 512:(c + 1) * 512],
                        in0=ho_ps[:], in1=recip_bc[:, c * 512:(c + 1) * 512],
                        op=OP.mult)
            return hoT

        def oproj_partial(h, hoT, woh, brow_sb):
            """acc_sb (+)= hoT.T @ w[h-block] (+ bias row on h==0)."""
            for tb in range(NT):
                sa_ps = psum_mm.tile([P, 512], f32, tag="mm")
                for kt in range(ND):
                    nc.tensor.matmul(
                        sa_ps[:],
                        lhsT=hoT[:, kt, tb * P:(tb + 1) * P],
                        rhs=woh[:, kt, :],
                        start=(kt == 0), stop=(h != 0 and kt == ND - 1))
                if h == 0:
                    nc.tensor.matmul(
                        sa_ps[:], lhsT=ones_row[:, 0:P], rhs=brow_sb[:],
                        start=False, stop=True)
                    nc.vector.tensor_copy(acc_sb[:, tb, :], sa_ps[:])
                else:
                    nc.vector.tensor_tensor(
                        out=acc_sb[:, tb, :], in0=acc_sb[:, tb, :],
                        in1=sa_ps[:], op=OP.add)

        # ============ phase 0+1: xT, self attention ===================
        with tc.tile_pool(name="xT", bufs=1) as xT_pool, \
             tc.tile_pool(name="qkv", bufs=1) as qkv_pool, \
             tc.tile_pool(name="hoTp", bufs=2) as hoT_pool, \
             tc.tile_pool(name="wstream", bufs=2) as wstream, \
             tc.tile_pool(name="rbc", bufs=2) as rbc_pool:
            xT = xT_pool.tile([P, ND, T], bf16)
            for tb in range(NT):
                transpose_to(x32_sb[:, tb, :], xT, ident_f, ND, tb, f32)
            qT = qkv_pool.tile([P, ND, T], bf16, tag="qT")
            kT = qkv_pool.tile([P, ND, T], bf16, tag="kT")
            v_sb = qkv_pool.tile([P, NS, D], bf16, tag="v")
            for h in range(H):
                wq_t = wstream.tile([P, ND, ND, P], bf16, tag="wq")
                nc.sync.dma_start(out=wq_t[:], in_=d_wq.ap()[h].rearrange(
                    "(kt p) (eb e) -> p kt eb e", p=P, e=P))
                wk_t = wstream.tile([P, ND, ND, P], bf16, tag="wk")
                nc.sync.dma_start(out=wk_t[:], in_=d_wk.ap()[h].rearrange(
                    "(kt p) (eb e) -> p kt eb e", p=P, e=P))
                wv_t = wstream.tile([P, ND, D], bf16, tag="wv")
                nc.sync.dma_start(out=wv_t[:], in_=d_wv.ap()[h].rearrange(
                    "(kt p) e -> p kt e", p=P))
                woh_t = wstream.tile([P, ND, D], bf16, tag="woh")
                nc.sync.dma_start(
                    out=woh_t[:],
                    in_=d_wo.ap()[h * D:(h + 1) * D].rearrange("(kt p) d -> p kt d", p=P))
                for eb in range(ND):
                    for c in range(NC2):
                        q_ps = psum_mm.tile([P, 512], f32, tag="mm")
                        for kt in range(ND):
                            nc.tensor.matmul(
                                q_ps[:], lhsT=wq_t[:, kt, eb, :],
                                rhs=xT[:, kt, c * 512:(c + 1) * 512],
                                start=(kt == 0), stop=(kt == ND - 1))
                        nc.vector.tensor_scalar_add(
                            qT[:, eb, c * 512:(c + 1) * 512], q_ps[:],
                            bq_sb[:, h * ND + eb:h * ND + eb + 1])
                        k_ps = psum_mm.tile([P, 512], f32, tag="mm")
                        for kt in range(ND):
                            nc.tensor.matmul(
                                k_ps[:], lhsT=wk_t[:, kt, eb, :],
                                rhs=xT[:, kt, c * 512:(c + 1) * 512],
                                start=(kt == 0), stop=(kt == ND - 1))
                        nc.vector.tensor_scalar_add(
                            kT[:, eb, c * 512:(c + 1) * 512], k_ps[:],
                            bk_sb[:, h * ND + eb:h * ND + eb + 1])
                for sb_ in range(NS):
                    v_ps = psum_mm.tile([P, 512], f32, tag="mm")
                    for kt in range(ND):
                        nc.tensor.matmul(
                            v_ps[:], lhsT=xT[:, kt, sb_ * P:(sb_ + 1) * P],
                            rhs=wv_t[:, kt, :],
                            start=(kt == 0), stop=(kt == ND - 1))
                    nc.vector.tensor_copy(v_sb[:, sb_, :], v_ps[:])
                hoT = attention(qT, kT, v_sb, tpad_sb, True, rbc_pool, hoT_pool)
                if h > 0:
                    oproj_partial(h - 1, prev[0], prev[1], bo_sb)
                prev = (hoT, woh_t)
            oproj_partial(H - 1, prev[0], prev[1], bo_sb)

        # ============ phase 2+3: LN1 -> x1/x1T, cross attention =======
        with tc.tile_pool(name="mem", bufs=1) as mem_pool, \
             tc.tile_pool(name="qkv2", bufs=1) as qkv2_pool, \
             tc.tile_pool(name="hoTp2", bufs=2) as hoT2_pool, \
             tc.tile_pool(name="wstream2", bufs=2) as wstream2, \
             tc.tile_pool(name="rbc2", bufs=2) as rbc2_pool:
            memk_sb = mem_pool.tile([P, NS, D], bf16, tag="memk")
            nc.sync.dma_start(out=memk_sb[:], in_=d_memk.ap().rearrange(
                "(st p) e -> p st e", p=P))
            memv_sb = mem_pool.tile([P, NS, D], bf16, tag="memv")
            nc.sync.dma_start(out=memv_sb[:], in_=d_memv.ap().rearrange(
                "(st p) e -> p st e", p=P))
            x1_sb = x1_pool.tile([P, NT, D], f32, tag="x1")
            x1T_sb = x1_pool.tile([P, ND, T], bf16, tag="x1T")
            memkT = mem_pool.tile([P, ND, S], bf16, tag="memkT")
            for tb in range(NT):
                layernorm(acc_sb[:, tb, :], x32_sb[:, tb, :], x1_sb[:, tb, :])
                transpose_to(x1_sb[:, tb, :], x1T_sb, ident_f, ND, tb, f32)
                # independent PE filler while the LN chain drains
                transpose_to(memk_sb[:, tb, :], memkT, ident_b, ND, tb, bf16)
            qmT = qkv2_pool.tile([P, ND, T], bf16, tag="qmT")
            for h in range(H):
                wqm_t = wstream2.tile([P, ND, ND, P], bf16, tag="wqm")
                nc.sync.dma_start(out=wqm_t[:], in_=d_wqm.ap()[h].rearrange(
                    "(kt p) (eb e) -> p kt eb e", p=P, e=P))
                womh_t = wstream2.tile([P, ND, D], bf16, tag="womh")
                nc.sync.dma_start(
                    out=womh_t[:],
                    in_=d_wom.ap()[h * D:(h + 1) * D].rearrange("(kt p) d -> p kt d", p=P))
                for eb in range(ND):
                    for c in range(NC2):
                        q_ps = psum_mm.tile([P, 512], f32, tag="mm")
                        for kt in range(ND):
                            nc.tensor.matmul(
                                q_ps[:], lhsT=wqm_t[:, kt, eb, :],
                                rhs=x1T_sb[:, kt, c * 512:(c + 1) * 512],
                                start=(kt == 0), stop=(kt == ND - 1))
                        nc.vector.tensor_scalar_add(
                            qmT[:, eb, c * 512:(c + 1) * 512], q_ps[:],
                            bqm_sb[:, h * ND + eb:h * ND + eb + 1])
                hoT = attention(qmT, memkT, memv_sb, spad_sb, False,
                                rbc2_pool, hoT2_pool)
                if h > 0:
                    oproj_partial(h - 1, prev[0], prev[1], bom_sb)
                prev = (hoT, womh_t)
            oproj_partial(H - 1, prev[0], prev[1], bom_sb)
        es_x32.close()

        # ============ phase 4+5: LN2 -> x2/x2T, FFN + LN3 =============
        with tc.tile_pool(name="ffn", bufs=1) as ffn_pool:
            w1_t = ffn_pool.tile([P, ND, NF, P], bf16, tag="w1")
            nc.sync.dma_start(out=w1_t[:], in_=d_w1.ap().rearrange(
                "(kt p) (fb f) -> p kt fb f", p=P, f=P))
            w2_t = ffn_pool.tile([P, NF, D], bf16, tag="w2")
            nc.sync.dma_start(out=w2_t[:], in_=d_w2.ap().rearrange(
                "(kt p) d -> p kt d", p=P))
            f1T = ffn_pool.tile([P, NF, T], bf16, tag="f1T")
            x2_sb = x2_pool.tile([P, NT, D], f32, tag="x2")
            x2T_sb = x2_pool.tile([P, ND, T], bf16, tag="x2T")

            def f1_chunk(c):
                for fb in range(NF):
                    f_ps = psum_mm.tile([P, 512], f32, tag="mm")
                    for kt in range(ND):
                        nc.tensor.matmul(
                            f_ps[:], lhsT=w1_t[:, kt, fb, :],
                            rhs=x2T_sb[:, kt, c * 512:(c + 1) * 512],
                            start=(kt == 0), stop=(kt == ND - 1))
                    nc.scalar.activation(
                        f1T[:, fb, c * 512:(c + 1) * 512], f_ps[:], AF.Relu,
                        bias=b1_sb[:, fb:fb + 1])

            for tb in range(NT):
                layernorm(acc_sb[:, tb, :], x1_sb[:, tb, :], x2_sb[:, tb, :])
                transpose_to(x2_sb[:, tb, :], x2T_sb, ident_f, ND, tb, f32)
                # start FFN chunk as soon as the t-blocks feeding it are done
                if tb == 3:
                    f1_chunk(0)
            f1_chunk(1)
            for tb in range(NT):
                o_ps = psum_mm.tile([P, 512], f32, tag="mm")
                for kt in range(NF):
                    nc.tensor.matmul(
                        o_ps[:], lhsT=f1T[:, kt, tb * P:(tb + 1) * P],
                        rhs=w2_t[:, kt, :], start=(kt == 0), stop=False)
                nc.tensor.matmul(
                    o_ps[:], lhsT=ones_row[:, 0:P], rhs=b2_sb[:],
                    start=False, stop=True)
                out_sb = small.tile([P, D], f32, tag="out_sb")
                layernorm(o_ps[:], x2_sb[:, tb, :], out_sb[:])
                nc.sync.dma_start(
                    out=d_out.ap().rearrange("(tb p) d -> p tb d", p=P)[:, tb, :],
                    in_=out_sb[:])
        es_x1.close()
        es_attn.close()
        es_x2.close()

    nc.compile()
    _CACHE["nc"] = nc
    return nc


def make_in_maps(inputs):
    import ml_dtypes

    bf = ml_dtypes.bfloat16
    f32 = np.float32

    def col_layout(bias_hd):  # [H, D] -> [P, H*ND], col h*ND+eb
        return np.ascontiguousarray(
            bias_hd.reshape(H, ND, P).transpose(2, 0, 1).reshape(P, H * ND)
        ).astype(f32)

    wo_f = np.asarray(inputs["Wo_self"], f32)
    bo_row = np.asarray(inputs["bo_self"], f32).copy()
    bv = np.asarray(inputs["bv_self"], f32)
    for h in range(H):
        bo_row += bv[h] @ wo_f[h * D:(h + 1) * D]

    shared = {
        "wq": np.asarray(inputs["Wq_self"], f32).astype(bf),
        "wk": np.asarray(inputs["Wk_self"], f32).astype(bf),
        "wv": np.asarray(inputs["Wv_self"], f32).astype(bf),
        "wqm": np.asarray(inputs["Wq_mem"], f32).astype(bf),
        "wo": wo_f.astype(bf),
        "wom": np.asarray(inputs["Wo_mem"], f32).astype(bf),
        "w1": np.asarray(inputs["W1"], f32).astype(bf),
        "w2": np.asarray(inputs["W2"], f32).astype(bf),
        "bq_c": col_layout(np.asarray(inputs["bq_self"], f32)),
        "bk_c": col_layout(np.asarray(inputs["bk_self"], f32)),
        "bqm_c": col_layout(np.asarray(inputs["bq_mem"], f32)),
        "b1_c": np.ascontiguousarray(
            np.asarray(inputs["b1"], f32).reshape(NF, P).T).astype(f32),
        "bo_row": bo_row.reshape(1, D).astype(bf),
        "bom_row": np.asarray(inputs["bo_mem"], f32).reshape(1, D).astype(bf),
        "b2_row": np.asarray(inputs["b2"], f32).reshape(1, D).astype(bf),
        # attT is [s, t]: transpose the causal diagonal block
        "diag": np.ascontiguousarray(
            np.asarray(inputs["tgt_subsq_mask"], f32)[:P, :P].T),
    }
    in_maps = []
    for b in range(B):
        m = dict(shared)
        m["x32"] = np.ascontiguousarray(np.asarray(inputs["x"], f32)[b])
        m["memk"] = np.asarray(inputs["mem_keys"], f32)[b].astype(bf)
        m["memv"] = np.asarray(inputs["mem_values"], f32)[b].astype(bf)
        m["tpad"] = np.ascontiguousarray(
            np.asarray(inputs["tgt_padding_mask"], f32)[b, :, 0].reshape(NS, P).T)
        m["spad"] = np.ascontiguousarray(
            np.asarray(inputs["src_padding_mask"], f32)[b, :, 0].reshape(NS, P).T)
        in_maps.append(m)
    return in_maps


def kernel(**inputs):
    from concourse.bass_utils import run_bass_kernel_spmd

    nc = _build()
    in_maps = make_in_maps(inputs)
    res = run_bass_kernel_spmd(nc, in_maps, list(range(B)))
    out = np.stack([np.asarray(res.results[i]["out"]) for i in range(B)])
    return out.astype(np.float32)



# revision 6
# speedup vs baseline: 1.5049x; 1.5049x over previous
"""Trainium2 Bass kernel for nn_DecoderLayer_19816979104174.

Data-parallel over batch: each of the 8 NeuronCores runs one batch element's
full decoder layer.

Key optimizations over the bf16 baseline:
  - Weight folding (host, exact in f32): M_h = Wq_h @ Wk_h^T folds the Q and
    K projections into one; G_h = Wv_h @ Wo_h folds the V projection into the
    output projection (softmax weights are applied directly to x). The
    bq-dependent score term (x @ Wk bq, per-key) folds into the exp bias; the
    bk-dependent term is softmax-invariant and dropped.
  - fp8(e4m3) DoubleRow matmuls (2x PE throughput) for all attention math.
    Weights are pre-scaled by 64 (std 0.009 -> 0.58) to clear e4m3's
    subnormal cliff; normalized attention outputs scaled by 16. All scales
    are powers of two and are undone exactly in the fp32 PSUM->SBUF flushes.
  - Output projections accumulate all heads in a single PSUM group per
    t-block (no per-head SBUF accumulation on the Vector engine).
  - x / mem_keys arrive pre-transposed/pre-quantized from the host.
  - FFN stays bf16: fp8 error there (~0.29-std signal through K=2048) would
    threaten the 2e-2 relative-error budget.
"""

import sys

sys.path.insert(0, "/opt/trn_rl_repo")
sys.path.insert(0, "/root/.axon_site/_ro/trn_rl_repo")

import numpy as np

B, T, S, D, H, F = 8, 1024, 1024, 512, 8, 2048
P = 128
NT, ND, NS, NF = T // P, D // P, S // P, F // P
NC2 = T // 512  # 512-wide t chunks
SCALE = 1.0 / float(np.sqrt(D))
LN_EPS = 1e-5

WSC_M = 64.0    # m8 = 64 * Wq@Wk^T
WSC_QM = 32.0   # wqm8 = 32 * Wq_mem  (qm values ~N(0,14.5) stay < 240)
WSC_O = 64.0    # g8 = 64 * Wv@Wo_h ; wom8 = 64 * Wo_mem
USC = 16.0      # uT = 16 * softmax-weighted sums (via ones2 = 1/16)
OSC = 1.0 / (WSC_O * USC)   # oproj PSUM flush scale = 2^-10
RSC = 1024.0    # cq8/cm8 bias-vector pre-scale
RINV = 1.0 / RSC

_CACHE = {}


def _build():
    if "nc" in _CACHE:
        return _CACHE["nc"]

    import concourse.tile as tile
    import concourse.mybir as mybir
    from concourse import bacc
    from concourse.masks import make_identity
    from contextlib import ExitStack

    bf16 = mybir.dt.bfloat16
    f32 = mybir.dt.float32
    fp8 = mybir.dt.float8e4
    AF = mybir.ActivationFunctionType
    OP = mybir.AluOpType
    DR = mybir.MatmulPerfMode.DoubleRow

    nc = bacc.Bacc("TRN2")

    # ---- DRAM I/O -----------------------------------------------------
    d_x32 = nc.dram_tensor("x32", [T, D], f32, kind="ExternalInput")
    d_x8 = nc.dram_tensor("x8", [T, D], fp8, kind="ExternalInput")
    d_xT8 = nc.dram_tensor("xT8", [D, T], fp8, kind="ExternalInput")
    d_m8 = nc.dram_tensor("m8", [H, D, D], fp8, kind="ExternalInput")
    d_g8 = nc.dram_tensor("g8", [H * D, D], fp8, kind="ExternalInput")
    d_cq8 = nc.dram_tensor("cq8", [P, ND, H], fp8, kind="ExternalInput")
    d_wqm8 = nc.dram_tensor("wqm8", [H, D, D], fp8, kind="ExternalInput")
    d_wom8 = nc.dram_tensor("wom8", [H * D, D], fp8, kind="ExternalInput")
    d_cm8 = nc.dram_tensor("cm8", [P, ND, H], fp8, kind="ExternalInput")
    d_w1 = nc.dram_tensor("w1", [D, F], bf16, kind="ExternalInput")
    d_w2 = nc.dram_tensor("w2", [F, D], bf16, kind="ExternalInput")
    d_b1 = nc.dram_tensor("b1_c", [P, NF], f32, kind="ExternalInput")
    d_bo = nc.dram_tensor("bo_row", [1, D], bf16, kind="ExternalInput")
    d_bom = nc.dram_tensor("bom_row", [1, D], bf16, kind="ExternalInput")
    d_b2 = nc.dram_tensor("b2_row", [1, D], bf16, kind="ExternalInput")
    d_memkT8 = nc.dram_tensor("memkT8", [D, S], fp8, kind="ExternalInput")
    d_memv8 = nc.dram_tensor("memv8", [S, D], fp8, kind="ExternalInput")
    d_tpad = nc.dram_tensor("tpad", [P, NS], f32, kind="ExternalInput")
    d_spad = nc.dram_tensor("spad", [P, NS], f32, kind="ExternalInput")
    d_diag = nc.dram_tensor("diag", [P, P], f32, kind="ExternalInput")
    d_out = nc.dram_tensor("out", [T, D], f32, kind="ExternalOutput")

    with tile.TileContext(nc) as tc, ExitStack() as ctx:
        const = ctx.enter_context(tc.tile_pool(name="const", bufs=1))
        small = ctx.enter_context(tc.tile_pool(name="small", bufs=2))
        psum_mm = ctx.enter_context(tc.tile_pool(name="psum_mm", bufs=4, space="PSUM"))
        psum_tr = ctx.enter_context(tc.tile_pool(name="psum_tr", bufs=2, space="PSUM"))
        psum_rs = ctx.enter_context(tc.tile_pool(name="psum_rs", bufs=2, space="PSUM"))

        # ---- constants / small inputs --------------------------------
        ident_f = const.tile([P, P], f32)
        make_identity(nc, ident_f)
        ones1 = const.tile([P, 1], fp8)
        nc.vector.memset(ones1[:], 1.0 / USC)
        ones_row = const.tile([1, P], bf16)
        nc.vector.memset(ones_row[:], 1.0)
        eps_t = const.tile([P, 1], f32)
        nc.vector.memset(eps_t[:], LN_EPS)
        diag_sb = const.tile([P, P], f32)
        nc.gpsimd.dma_start(out=diag_sb[:], in_=d_diag.ap())
        tpad_sb = const.tile([P, NS], f32)
        nc.gpsimd.dma_start(out=tpad_sb[:], in_=d_tpad.ap())
        spad_sb = const.tile([P, NS], f32)
        nc.gpsimd.dma_start(out=spad_sb[:], in_=d_spad.ap())
        cq_sb = const.tile([P, ND, H], fp8)
        nc.gpsimd.dma_start(out=cq_sb[:], in_=d_cq8.ap())
        cm_sb = const.tile([P, ND, H], fp8)
        nc.gpsimd.dma_start(out=cm_sb[:], in_=d_cm8.ap())
        b1_sb = const.tile([P, NF], f32)
        nc.gpsimd.dma_start(out=b1_sb[:], in_=d_b1.ap())
        bo_sb = const.tile([1, D], bf16)
        nc.gpsimd.dma_start(out=bo_sb[:], in_=d_bo.ap())
        bom_sb = const.tile([1, D], bf16)
        nc.gpsimd.dma_start(out=bom_sb[:], in_=d_bom.ap())
        b2_sb = const.tile([1, D], bf16)
        nc.gpsimd.dma_start(out=b2_sb[:], in_=d_b2.ap())

        # ---- pools with phase-scoped lifetimes (LIFO close order) ----
        es_x2 = ExitStack()     # x2/x2T: FFN phase
        x2_pool = es_x2.enter_context(tc.tile_pool(name="x2p", bufs=1))
        es_attn = ExitStack()   # expT + uTall: self + cross attention
        attn_pool = es_attn.enter_context(tc.tile_pool(name="attn", bufs=1))
        es_x1 = ExitStack()     # x1/x1T: cross phase
        x1_pool = es_x1.enter_context(tc.tile_pool(name="x1p", bufs=1))
        es_x32 = ExitStack()    # x (f32/fp8/T), self radd: self phase
        x32_pool = es_x32.enter_context(tc.tile_pool(name="x32p", bufs=1))

        expT = attn_pool.tile([P, NS, T], fp8, tag="expT")
        nc.gpsimd.memset(expT[:], 0.0)
        uTall = attn_pool.tile([P, H, ND, T], fp8, tag="uTall")
        x32_sb = x32_pool.tile([P, NT, D], f32, tag="x32")
        nc.sync.dma_start(
            out=x32_sb[:], in_=d_x32.ap().rearrange("(tb p) d -> p tb d", p=P))
        x8_sb = x32_pool.tile([P, NT, D], fp8, tag="x8")
        nc.sync.dma_start(
            out=x8_sb[:], in_=d_x8.ap().rearrange("(tb p) d -> p tb d", p=P))
        xT8 = x32_pool.tile([P, ND, T], fp8, tag="xT8")
        nc.sync.dma_start(
            out=xT8[:], in_=d_xT8.ap().rearrange("(eb p) t -> p eb t", p=P))

        def transpose_to(src_ap, dstT, tb):
            for dt in range(ND):
                tr_ps = psum_tr.tile([P, P], f32, tag="tr")
                nc.tensor.transpose(
                    tr_ps[:], src_ap[:, dt * P:(dt + 1) * P], ident_f[:])
                nc.vector.tensor_copy(dstT[:, dt, tb * P:(tb + 1) * P], tr_ps[:])

        def layernorm(src_ap, scale, resid_ap, dst_ap):
            res = small.tile([P, D], f32, tag="ln_res")
            if scale is None:
                nc.vector.tensor_tensor(
                    out=res[:], in0=src_ap, in1=resid_ap, op=OP.add)
            else:
                nc.vector.scalar_tensor_tensor(
                    out=res[:], in0=src_ap, scalar=scale, in1=resid_ap,
                    op0=OP.mult, op1=OP.add)
            stats = small.tile([P, 6], f32, tag="ln_stats")
            nc.vector.bn_stats(stats[:], res[:])
            mv = small.tile([P, 2], f32, tag="ln_mv")
            nc.vector.bn_aggr(mv[:], stats[:])
            std = small.tile([P, 1], f32, tag="ln_std")
            nc.scalar.activation(std[:], mv[:, 1:2], AF.Sqrt, bias=eps_t[:])
            istd = small.tile([P, 1], f32, tag="ln_istd")
            nc.vector.reciprocal(istd[:], std[:])
            nc.vector.tensor_scalar(
                out=dst_ap, in0=res[:], scalar1=mv[:, 0:1], scalar2=istd[:],
                op0=OP.subtract, op1=OP.mult)

        def build_radd(lhsT_sb, c_sb, pad_sb, radd):
            """radd[:, j, h] = pad[:, j] + RINV * sum_d lhsT[d, s] c[d, h]."""
            for j in range(NS):
                r_ps = psum_mm.tile([P, 512], f32, tag="mm")
                for kp in range(ND // 2):
                    nc.tensor.matmul(
                        r_ps[:, 0:H], lhsT=lhsT_sb[:, 2 * kp:2 * kp + 2,
                                                   j * P:(j + 1) * P],
                        rhs=c_sb[:, 2 * kp:2 * kp + 2, :], perf_mode=DR,
                        start=(kp == 0), stop=(kp == ND // 2 - 1))
                nc.vector.tensor_scalar(
                    out=radd[:, j, :], in0=r_ps[:, 0:H], scalar1=RINV,
                    scalar2=pad_sb[:, j:j + 1], op0=OP.mult, op1=OP.add)

        def proj_qT(w_t, src_T8, dstT8):
            """dstT8[e, t] = sum_d w[d, e] src[d, t]   (both fp8, DoubleRow)."""
            for eb in range(ND):
                for c in range(NC2):
                    ps = psum_mm.tile([P, 512], f32, tag="mm")
                    for kp in range(ND // 2):
                        nc.tensor.matmul(
                            ps[:], lhsT=w_t[:, 2 * kp:2 * kp + 2, eb, :],
                            rhs=src_T8[:, 2 * kp:2 * kp + 2,
                                       c * 512:(c + 1) * 512],
                            perf_mode=DR, start=(kp == 0),
                            stop=(kp == ND // 2 - 1))
                    nc.vector.tensor_copy(
                        dstT8[:, eb, c * 512:(c + 1) * 512], ps[:])

        def rowsum_chunk(c, jmax, recipT, recip_bc):
            sl = slice(c * 512, (c + 1) * 512)
            rs_ps = psum_rs.tile([1, 512], f32, tag="rs")
            for j in range(jmax):
                nc.tensor.matmul(
                    rs_ps[:], lhsT=ones1[:], rhs=expT[:, j, sl],
                    start=(j == 0), stop=(j == jmax - 1))
            nc.vector.reciprocal(recipT[:, sl], rs_ps[:])
            nc.gpsimd.partition_broadcast(recip_bc[:, sl], recipT[:, sl])

        def u_chunk(h, c, jmax, v_sb, recip_bc):
            sl = slice(c * 512, (c + 1) * 512)
            for eb in range(ND):
                ps = psum_mm.tile([P, 512], f32, tag="mm")
                for kp in range(jmax // 2):
                    nc.tensor.matmul(
                        ps[:], lhsT=v_sb[:, 2 * kp:2 * kp + 2,
                                         eb * P:(eb + 1) * P],
                        rhs=expT[:, 2 * kp:2 * kp + 2, sl], perf_mode=DR,
                        start=(kp == 0), stop=(kp == jmax // 2 - 1))
                nc.vector.tensor_tensor(
                    out=uTall[:, h, eb, sl], in0=ps[:], in1=recip_bc[:, sl],
                    op=OP.mult)

        def oproj_ln(wall, brow_sb, resid_sb, dst_sb, dstT):
            """Per t-block: accumulate all heads' uT @ W in one PSUM group,
            add the bias row, then LN(psum*OSC + resid) -> dst (+transpose)."""
            prev = -1
            for tb in range(NT):
                ps = psum_mm.tile([P, 512], f32, tag="mm")
                for h in range(H):
                    for kp in range(ND // 2):
                        nc.tensor.matmul(
                            ps[:], lhsT=uTall[:, h, 2 * kp:2 * kp + 2,
                                              tb * P:(tb + 1) * P],
                            rhs=wall[:, h, 2 * kp:2 * kp + 2, :],
                            perf_mode=DR, start=(h == 0 and kp == 0),
                            stop=False)
                nc.tensor.matmul(
                    ps[:], lhsT=ones_row[:, 0:P], rhs=brow_sb[:],
                    start=False, stop=True)
                layernorm(ps[:], OSC, resid_sb[:, tb, :], dst_sb[:, tb, :])
                if prev >= 0:
                    transpose_to(dst_sb[:, prev, :], dstT, prev)
                prev = tb
            transpose_to(dst_sb[:, prev, :], dstT, prev)

        # ============ phase 1: self attention =========================
        radd_s = x32_pool.tile([P, NS, H], f32, tag="radd_s")
        build_radd(xT8, cq_sb, tpad_sb, radd_s)

        with tc.tile_pool(name="gallp", bufs=1) as gall_pool, \
             tc.tile_pool(name="qkv", bufs=1) as qkv_pool, \
             tc.tile_pool(name="wstream", bufs=2) as wstream, \
             tc.tile_pool(name="rbc", bufs=2) as rbc_pool:
            gall = gall_pool.tile([P, H, ND, D], fp8)
            nc.sync.dma_start(out=gall[:], in_=d_g8.ap().rearrange(
                "(h kt p) e -> p h kt e", p=P, h=H))
            qMT = qkv_pool.tile([P, ND, T], fp8, tag="qMT")

            def att_block(j, c, radd, h):
                lo = max(j * P, c * 512)
                w = (c + 1) * 512 - lo
                ps = psum_mm.tile([P, 512], f32, tag="mm")
                for kp in range(ND // 2):
                    nc.tensor.matmul(
                        ps[:, :w],
                        lhsT=xT8[:, 2 * kp:2 * kp + 2, j * P:(j + 1) * P],
                        rhs=qMT[:, 2 * kp:2 * kp + 2, lo:(c + 1) * 512],
                        perf_mode=DR, start=(kp == 0),
                        stop=(kp == ND // 2 - 1))
                if lo == j * P:
                    nc.vector.tensor_tensor(
                        out=ps[:, 0:P], in0=ps[:, 0:P], in1=diag_sb[:],
                        op=OP.add)
                nc.scalar.activation(
                    expT[:, j, lo:(c + 1) * 512], ps[:, :w], AF.Exp,
                    bias=radd[:, j, h:h + 1], scale=SCALE / WSC_M)

            for h in range(H):
                m_t = wstream.tile([P, ND, ND, P], fp8, tag="m")
                nc.sync.dma_start(out=m_t[:], in_=d_m8.ap()[h].rearrange(
                    "(kt p) (eb e) -> p kt eb e", p=P, e=P))
                proj_qT(m_t, xT8, qMT)
                recipT = rbc_pool.tile([1, T], f32, tag="recipT")
                recip_bc = rbc_pool.tile([P, T], f32, tag="recip_bc")
                for j in range(4):
                    att_block(j, 0, radd_s, h)
                    att_block(j, 1, radd_s, h)
                rowsum_chunk(0, 4, recipT, recip_bc)
                u_chunk(h, 0, 4, x8_sb, recip_bc)
                for j in range(4, NS):
                    att_block(j, 1, radd_s, h)
                rowsum_chunk(1, NS, recipT, recip_bc)
                u_chunk(h, 1, NS, x8_sb, recip_bc)

            x1_sb = x1_pool.tile([P, NT, D], f32, tag="x1")
            x1T8 = x1_pool.tile([P, ND, T], fp8, tag="x1T")
            oproj_ln(gall, bo_sb, x32_sb, x1_sb, x1T8)
        es_x32.close()

        # ============ phase 2: cross attention ========================
        with tc.tile_pool(name="mem", bufs=1) as mem_pool, \
             tc.tile_pool(name="qkv2", bufs=1) as qkv2_pool, \
             tc.tile_pool(name="wstream2", bufs=2) as wstream2, \
             tc.tile_pool(name="rbc2", bufs=2) as rbc2_pool:
            memkT8 = mem_pool.tile([P, ND, S], fp8, tag="memkT")
            nc.sync.dma_start(out=memkT8[:], in_=d_memkT8.ap().rearrange(
                "(eb p) s -> p eb s", p=P))
            memv8 = mem_pool.tile([P, NS, D], fp8, tag="memv")
            nc.sync.dma_start(out=memv8[:], in_=d_memv8.ap().rearrange(
                "(st p) e -> p st e", p=P))
            womall = mem_pool.tile([P, H, ND, D], fp8, tag="womall")
            nc.sync.dma_start(out=womall[:], in_=d_wom8.ap().rearrange(
                "(h kt p) e -> p h kt e", p=P, h=H))
            radd_m = mem_pool.tile([P, NS, H], f32, tag="radd_m")
            build_radd(memkT8, cm_sb, spad_sb, radd_m)
            qmT = qkv2_pool.tile([P, ND, T], fp8, tag="qmT")

            for h in range(H):
                wqm_t = wstream2.tile([P, ND, ND, P], fp8, tag="wqm")
                nc.sync.dma_start(out=wqm_t[:], in_=d_wqm8.ap()[h].rearrange(
                    "(kt p) (eb e) -> p kt eb e", p=P, e=P))
                proj_qT(wqm_t, x1T8, qmT)
                recipT = rbc2_pool.tile([1, T], f32, tag="recipT")
                recip_bc = rbc2_pool.tile([P, T], f32, tag="recip_bc")
                for c in range(NC2):
                    for j in range(NS):
                        ps = psum_mm.tile([P, 512], f32, tag="mm")
                        for kp in range(ND // 2):
                            nc.tensor.matmul(
                                ps[:],
                                lhsT=memkT8[:, 2 * kp:2 * kp + 2,
                                            j * P:(j + 1) * P],
                                rhs=qmT[:, 2 * kp:2 * kp + 2,
                                        c * 512:(c + 1) * 512],
                                perf_mode=DR, start=(kp == 0),
                                stop=(kp == ND // 2 - 1))
                        nc.scalar.activation(
                            expT[:, j, c * 512:(c + 1) * 512], ps[:], AF.Exp,
                            bias=radd_m[:, j, h:h + 1], scale=SCALE / WSC_QM)
                    rowsum_chunk(c, NS, recipT, recip_bc)
                    u_chunk(h, c, NS, memv8, recip_bc)

            x2_sb = x2_pool.tile([P, NT, D], f32, tag="x2")
            x2T = x2_pool.tile([P, ND, T], bf16, tag="x2T")
            oproj_ln(womall, bom_sb, x1_sb, x2_sb, x2T)
        es_x1.close()
        es_attn.close()

        # ============ phase 3: FFN + LN3 (bf16) =======================
        with tc.tile_pool(name="ffn", bufs=1) as ffn_pool:
            w1_t = ffn_pool.tile([P, ND, NF, P], bf16, tag="w1")
            nc.sync.dma_start(out=w1_t[:], in_=d_w1.ap().rearrange(
                "(kt p) (fb f) -> p kt fb f", p=P, f=P))
            w2_t = ffn_pool.tile([P, NF, D], bf16, tag="w2")
            nc.sync.dma_start(out=w2_t[:], in_=d_w2.ap().rearrange(
                "(kt p) d -> p kt d", p=P))
            f1T = ffn_pool.tile([P, NF, T], bf16, tag="f1T")

            def f1_chunk(c):
                for fb in range(NF):
                    f_ps = psum_mm.tile([P, 512], f32, tag="mm")
                    for kt in range(ND):
                        nc.tensor.matmul(
                            f_ps[:], lhsT=w1_t[:, kt, fb, :],
                            rhs=x2T[:, kt, c * 512:(c + 1) * 512],
                            start=(kt == 0), stop=(kt == ND - 1))
                    nc.scalar.activation(
                        f1T[:, fb, c * 512:(c + 1) * 512], f_ps[:], AF.Relu,
                        bias=b1_sb[:, fb:fb + 1])

            f1_chunk(0)
            f1_chunk(1)
            for tb in range(NT):
                o_ps = psum_mm.tile([P, 512], f32, tag="mm")
                for kt in range(NF):
                    nc.tensor.matmul(
                        o_ps[:], lhsT=f1T[:, kt, tb * P:(tb + 1) * P],
                        rhs=w2_t[:, kt, :], start=(kt == 0), stop=False)
                nc.tensor.matmul(
                    o_ps[:], lhsT=ones_row[:, 0:P], rhs=b2_sb[:],
                    start=False, stop=True)
                out_sb = small.tile([P, D], f32, tag="out_sb")
                layernorm(o_ps[:], None, x2_sb[:, tb, :], out_sb[:])
                nc.sync.dma_start(
                    out=d_out.ap().rearrange("(tb p) d -> p tb d", p=P)[:, tb, :],
                    in_=out_sb[:])
        es_x2.close()

    nc.compile()
    _CACHE["nc"] = nc
    return nc


def make_in_maps(inputs):
    import ml_dtypes

    bf = ml_dtypes.bfloat16
    f8 = ml_dtypes.float8_e4m3
    f32 = np.float32

    wq = np.asarray(inputs["Wq_self"], f32)
    wk = np.asarray(inputs["Wk_self"], f32)
    wv = np.asarray(inputs["Wv_self"], f32)
    wo = np.asarray(inputs["Wo_self"], f32)
    bq = np.asarray(inputs["bq_self"], f32)
    bv = np.asarray(inputs["bv_self"], f32)
    wqm = np.asarray(inputs["Wq_mem"], f32)
    wom = np.asarray(inputs["Wo_mem"], f32)
    bqm = np.asarray(inputs["bq_mem"], f32)

    # fold Q/K into M = Wq Wk^T, V into G = Wv Wo_h
    m = np.matmul(wq, wk.transpose(0, 2, 1))                 # [H, D, D]
    g = np.matmul(wv, wo.reshape(H, D, D))                   # [H, D, D]
    # per-key score bias: x @ (Wk bq); the Wq bk term is softmax-invariant
    cq = np.einsum('hde,he->dh', wk, bq) * (RSC * SCALE)     # [D, H]
    cm = bqm.T * (RSC * SCALE)                               # [D, H]

    bo_row = np.asarray(inputs["bo_self"], f32).copy()
    for h in range(H):
        bo_row += bv[h] @ wo[h * D:(h + 1) * D]

    def colh(a):  # [D, H] -> [P, ND, H]
        return np.ascontiguousarray(a.reshape(ND, P, H).transpose(1, 0, 2))

    shared = {
        "m8": (m * WSC_M).astype(f8),
        "g8": (g * WSC_O).reshape(H * D, D).astype(f8),
        "cq8": colh(cq).astype(f8),
        "wqm8": (wqm * WSC_QM).astype(f8),
        "wom8": (wom * WSC_O).astype(f8),
        "cm8": colh(cm).astype(f8),
        "w1": np.asarray(inputs["W1"], f32).astype(bf),
        "w2": np.asarray(inputs["W2"], f32).astype(bf),
        "b1_c": np.ascontiguousarray(
            np.asarray(inputs["b1"], f32).reshape(NF, P).T).astype(f32),
        "bo_row": (bo_row * (WSC_O * USC)).reshape(1, D).astype(bf),
        "bom_row": (np.asarray(inputs["bo_mem"], f32)
                    * (WSC_O * USC)).reshape(1, D).astype(bf),
        "b2_row": np.asarray(inputs["b2"], f32).reshape(1, D).astype(bf),
        # attT is [s, t]: transpose the causal diagonal block
        "diag": np.ascontiguousarray(
            np.asarray(inputs["tgt_subsq_mask"], f32)[:P, :P].T),
    }
    in_maps = []
    for b in range(B):
        xb = np.asarray(inputs["x"], f32)[b]
        mk = np.asarray(inputs["mem_keys"], f32)[b]
        mvv = np.asarray(inputs["mem_values"], f32)[b]
        mm = dict(shared)
        mm["x32"] = np.ascontiguousarray(xb)
        mm["x8"] = xb.astype(f8)
        mm["xT8"] = np.ascontiguousarray(xb.T).astype(f8)
        mm["memkT8"] = np.ascontiguousarray(mk.T).astype(f8)
        mm["memv8"] = mvv.astype(f8)
        mm["tpad"] = np.ascontiguousarray(
            np.asarray(inputs["tgt_padding_mask"], f32)[b, :, 0].reshape(NS, P).T)
        mm["spad"] = np.ascontiguousarray(
            np.asarray(inputs["src_padding_mask"], f32)[b, :, 0].reshape(NS, P).T)
        in_maps.append(mm)
    return in_maps


def kernel(**inputs):
    from concourse.bass_utils import run_bass_kernel_spmd

    nc = _build()
    in_maps = make_in_maps(inputs)
    res = run_bass_kernel_spmd(nc, in_maps, list(range(B)))
    out = np.stack([np.asarray(res.results[i]["out"]) for i in range(B)])
    return out.astype(np.float32)


# revision 13
# speedup vs baseline: 1.8160x; 1.2067x over previous
"""Trainium2 Bass kernel for nn_DecoderLayer_19816979104174.

Data-parallel over batch: each of the 8 NeuronCores runs one batch element's
full decoder layer.

Key optimizations over the bf16 baseline:
  - Weight folding (host, exact in f32): M_h = Wq_h @ Wk_h^T folds the Q and
    K projections into one; G_h = Wv_h @ Wo_h folds the V projection into the
    output projection (softmax weights are applied directly to x). The
    bq-dependent score term (x @ Wk bq, per-key) folds into the exp bias; the
    bk-dependent term is softmax-invariant and dropped.
  - fp8(e4m3) DoubleRow matmuls (2x PE throughput) for all attention math.
    Weights are pre-scaled by 64 (std 0.009 -> 0.58) to clear e4m3's
    subnormal cliff; normalized attention outputs scaled by 16. All scales
    are powers of two and are undone exactly in the fp32 PSUM->SBUF flushes.
  - Output projections accumulate all heads in a single PSUM group per
    t-block (no per-head SBUF accumulation on the Vector engine).
  - x / mem_keys arrive pre-transposed/pre-quantized from the host.
  - FFN stays bf16: fp8 error there (~0.29-std signal through K=2048) would
    threaten the 2e-2 relative-error budget.
"""

import sys

sys.path.insert(0, "/opt/trn_rl_repo")
sys.path.insert(0, "/root/.axon_site/_ro/trn_rl_repo")

import numpy as np

B, T, S, D, H, F = 8, 1024, 1024, 512, 8, 2048
P = 128
NT, ND, NS, NF = T // P, D // P, S // P, F // P
NC2 = T // 512  # 512-wide t chunks
SCALE = 1.0 / float(np.sqrt(D))
LN_EPS = 1e-5

WSC_M = 64.0    # m8 = 64 * Wq@Wk^T
WSC_QM = 32.0   # wqm8 = 32 * Wq_mem  (qm values ~N(0,14.5) stay < 240)
WSC_O = 64.0    # g8 = 64 * Wv@Wo_h ; wom8 = 64 * Wo_mem
USC = 16.0      # uT = 16 * softmax-weighted sums (via ones2 = 1/16)
OSC = 1.0 / (WSC_O * USC)   # oproj PSUM flush scale = 2^-10
RSC = 1024.0    # cq8/cm8 bias-vector pre-scale
RINV = 1.0 / RSC

_CACHE = {}


def _build():
    if "nc" in _CACHE:
        return _CACHE["nc"]

    import concourse.tile as tile
    import concourse.mybir as mybir
    from concourse import bacc
    from concourse.masks import make_identity
    from contextlib import ExitStack

    bf16 = mybir.dt.bfloat16
    f32 = mybir.dt.float32
    fp8 = mybir.dt.float8e4
    AF = mybir.ActivationFunctionType
    OP = mybir.AluOpType
    DR = mybir.MatmulPerfMode.DoubleRow

    nc = bacc.Bacc("TRN2")

    # ---- DRAM I/O -----------------------------------------------------
    d_x32 = nc.dram_tensor("x32", [T, D], f32, kind="ExternalInput")
    d_x8 = nc.dram_tensor("x8", [T, D], fp8, kind="ExternalInput")
    d_xT8 = nc.dram_tensor("xT8", [D, T], fp8, kind="ExternalInput")
    d_m8 = nc.dram_tensor("m8", [H, D, D], fp8, kind="ExternalInput")
    d_g8 = nc.dram_tensor("g8", [H * D, D], fp8, kind="ExternalInput")
    d_cq8 = nc.dram_tensor("cq8", [P, ND, H], fp8, kind="ExternalInput")
    d_wqm8 = nc.dram_tensor("wqm8", [H, D, D], fp8, kind="ExternalInput")
    d_wom8 = nc.dram_tensor("wom8", [H * D, D], fp8, kind="ExternalInput")
    d_cm8 = nc.dram_tensor("cm8", [P, ND, H], fp8, kind="ExternalInput")
    d_w1 = nc.dram_tensor("w1", [D, F], bf16, kind="ExternalInput")
    d_w2 = nc.dram_tensor("w2", [F, D], bf16, kind="ExternalInput")
    d_b1 = nc.dram_tensor("b1_c", [P, NF], f32, kind="ExternalInput")
    d_bo = nc.dram_tensor("bo_row", [1, D], bf16, kind="ExternalInput")
    d_bom = nc.dram_tensor("bom_row", [1, D], bf16, kind="ExternalInput")
    d_b2 = nc.dram_tensor("b2_row", [1, D], bf16, kind="ExternalInput")
    d_memkT8 = nc.dram_tensor("memkT8", [D, S], fp8, kind="ExternalInput")
    d_memv8 = nc.dram_tensor("memv8", [S, D], fp8, kind="ExternalInput")
    d_tpad = nc.dram_tensor("tpad", [P, NS], f32, kind="ExternalInput")
    d_spad = nc.dram_tensor("spad", [P, NS], f32, kind="ExternalInput")
    d_diag = nc.dram_tensor("diag", [P, P], f32, kind="ExternalInput")
    d_out = nc.dram_tensor("out", [T, D], f32, kind="ExternalOutput")

    with tile.TileContext(nc) as tc, ExitStack() as ctx:
        const = ctx.enter_context(tc.tile_pool(name="const", bufs=1))
        small = ctx.enter_context(tc.tile_pool(name="small", bufs=2))
        psum_mm = ctx.enter_context(tc.tile_pool(name="psum_mm", bufs=4, space="PSUM"))
        psum_tr = ctx.enter_context(tc.tile_pool(name="psum_tr", bufs=2, space="PSUM"))
        psum_rs = ctx.enter_context(tc.tile_pool(name="psum_rs", bufs=2, space="PSUM"))

        # ---- constants / small inputs --------------------------------
        ident_f = const.tile([P, P], f32)
        make_identity(nc, ident_f)
        ones1 = const.tile([P, 1], fp8)
        nc.vector.memset(ones1[:], 1.0 / USC)
        ones_row = const.tile([1, P], bf16)
        nc.vector.memset(ones_row[:], 1.0)
        eps_t = const.tile([P, 1], f32)
        nc.vector.memset(eps_t[:], LN_EPS)
        diag_sb = const.tile([P, P], f32)
        nc.gpsimd.dma_start(out=diag_sb[:], in_=d_diag.ap())
        tpad_sb = const.tile([P, NS], f32)
        nc.gpsimd.dma_start(out=tpad_sb[:], in_=d_tpad.ap())
        spad_sb = const.tile([P, NS], f32)
        nc.gpsimd.dma_start(out=spad_sb[:], in_=d_spad.ap())
        cq_sb = const.tile([P, ND, H], fp8)
        nc.gpsimd.dma_start(out=cq_sb[:], in_=d_cq8.ap())
        cm_sb = const.tile([P, ND, H], fp8)
        nc.gpsimd.dma_start(out=cm_sb[:], in_=d_cm8.ap())
        b1_sb = const.tile([P, NF], f32)
        nc.gpsimd.dma_start(out=b1_sb[:], in_=d_b1.ap())
        bo_sb = const.tile([1, D], bf16)
        nc.gpsimd.dma_start(out=bo_sb[:], in_=d_bo.ap())
        bom_sb = const.tile([1, D], bf16)
        nc.gpsimd.dma_start(out=bom_sb[:], in_=d_bom.ap())
        b2_sb = const.tile([1, D], bf16)
        nc.gpsimd.dma_start(out=b2_sb[:], in_=d_b2.ap())

        # ---- pools with phase-scoped lifetimes (LIFO close order) ----
        es_x2 = ExitStack()     # x2/x2T: FFN phase
        x2_pool = es_x2.enter_context(tc.tile_pool(name="x2p", bufs=1))
        es_attn = ExitStack()   # expT + uTall: self + cross attention
        attn_pool = es_attn.enter_context(tc.tile_pool(name="attn", bufs=1))
        es_x1 = ExitStack()     # x1/x1T: cross phase
        x1_pool = es_x1.enter_context(tc.tile_pool(name="x1p", bufs=1))
        es_x32 = ExitStack()    # x (f32/fp8/T), self radd: self phase
        x32_pool = es_x32.enter_context(tc.tile_pool(name="x32p", bufs=1))

        expT = attn_pool.tile([P, NS, T], fp8, tag="expT")
        nc.gpsimd.memset(expT[:], 0.0)
        uTall = attn_pool.tile([P, H, ND, T], fp8, tag="uTall")
        x32_sb = x32_pool.tile([P, NT, D], f32, tag="x32")
        nc.sync.dma_start(
            out=x32_sb[:], in_=d_x32.ap().rearrange("(tb p) d -> p tb d", p=P))
        x8_sb = x32_pool.tile([P, NT, D], fp8, tag="x8")
        nc.sync.dma_start(
            out=x8_sb[:], in_=d_x8.ap().rearrange("(tb p) d -> p tb d", p=P))
        xT8 = x32_pool.tile([P, ND, T], fp8, tag="xT8")
        nc.sync.dma_start(
            out=xT8[:], in_=d_xT8.ap().rearrange("(eb p) t -> p eb t", p=P))

        def transpose_to(src_ap, dstT, tb):
            for dt in range(ND):
                tr_ps = psum_tr.tile([P, P], f32, tag="tr")
                nc.tensor.transpose(
                    tr_ps[:], src_ap[:, dt * P:(dt + 1) * P], ident_f[:])
                nc.vector.tensor_copy(dstT[:, dt, tb * P:(tb + 1) * P], tr_ps[:])

        def layernorm(src_ap, scale, resid_ap, dst_ap):
            res = small.tile([P, D], f32, tag="ln_res")
            if scale is None:
                nc.vector.tensor_tensor(
                    out=res[:], in0=src_ap, in1=resid_ap, op=OP.add)
            else:
                nc.vector.scalar_tensor_tensor(
                    out=res[:], in0=src_ap, scalar=scale, in1=resid_ap,
                    op0=OP.mult, op1=OP.add)
            stats = small.tile([P, 6], f32, tag="ln_stats")
            nc.vector.bn_stats(stats[:], res[:])
            mv = small.tile([P, 2], f32, tag="ln_mv")
            nc.vector.bn_aggr(mv[:], stats[:])
            std = small.tile([P, 1], f32, tag="ln_std")
            nc.scalar.activation(std[:], mv[:, 1:2], AF.Sqrt, bias=eps_t[:])
            istd = small.tile([P, 1], f32, tag="ln_istd")
            nc.vector.reciprocal(istd[:], std[:])
            nc.vector.tensor_scalar(
                out=dst_ap, in0=res[:], scalar1=mv[:, 0:1], scalar2=istd[:],
                op0=OP.subtract, op1=OP.mult)

        def build_radd(lhsT_sb, c_sb, pad_sb, radd):
            """radd[:, j, h] = pad[:, j] + RINV * sum_d lhsT[d, s] c[d, h]."""
            for j in range(NS):
                r_ps = psum_mm.tile([P, 512], f32, tag="mm")
                for kp in range(ND // 2):
                    nc.tensor.matmul(
                        r_ps[:, 0:H], lhsT=lhsT_sb[:, 2 * kp:2 * kp + 2,
                                                   j * P:(j + 1) * P],
                        rhs=c_sb[:, 2 * kp:2 * kp + 2, :], perf_mode=DR,
                        start=(kp == 0), stop=(kp == ND // 2 - 1))
                nc.vector.tensor_scalar(
                    out=radd[:, j, :], in0=r_ps[:, 0:H], scalar1=RINV,
                    scalar2=pad_sb[:, j:j + 1], op0=OP.mult, op1=OP.add)

        def proj_qT(w_t, src_T8, dstT8):
            """dstT8[e, t] = sum_d w[d, e] src[d, t]   (both fp8, DoubleRow)."""
            for eb in range(ND):
                for c in range(NC2):
                    ps = psum_mm.tile([P, 512], f32, tag="mm")
                    for kp in range(ND // 2):
                        nc.tensor.matmul(
                            ps[:], lhsT=w_t[:, 2 * kp:2 * kp + 2, eb, :],
                            rhs=src_T8[:, 2 * kp:2 * kp + 2,
                                       c * 512:(c + 1) * 512],
                            perf_mode=DR, start=(kp == 0),
                            stop=(kp == ND // 2 - 1))
                    nc.scalar.activation(
                        dstT8[:, eb, c * 512:(c + 1) * 512], ps[:], AF.Copy)

        def rowsum_chunk(c, jmax, recipT, recip_bc):
            sl = slice(c * 512, (c + 1) * 512)
            rs_ps = psum_rs.tile([1, 512], f32, tag="rs")
            for j in range(jmax):
                nc.tensor.matmul(
                    rs_ps[:], lhsT=ones1[:], rhs=expT[:, j, sl],
                    start=(j == 0), stop=(j == jmax - 1))
            nc.vector.reciprocal(recipT[:, sl], rs_ps[:])
            nc.gpsimd.partition_broadcast(recip_bc[:, sl], recipT[:, sl])

        def u_chunk(h, c, jmax, v_sb, recip_bc):
            sl = slice(c * 512, (c + 1) * 512)
            for eb in range(ND):
                ps = psum_mm.tile([P, 512], f32, tag="mm")
                for kp in range(jmax // 2):
                    nc.tensor.matmul(
                        ps[:], lhsT=v_sb[:, 2 * kp:2 * kp + 2,
                                         eb * P:(eb + 1) * P],
                        rhs=expT[:, 2 * kp:2 * kp + 2, sl], perf_mode=DR,
                        start=(kp == 0), stop=(kp == jmax // 2 - 1))
                nc.vector.tensor_tensor(
                    out=uTall[:, h, eb, sl], in0=ps[:], in1=recip_bc[:, sl],
                    op=OP.mult)

        def oproj_ln(wall, brow_sb, resid_sb, dst_sb, dstT):
            """Per t-block: accumulate all heads' uT @ W in one PSUM group,
            add the bias row, then LN(psum*OSC + resid) -> dst (+transpose)."""
            prev = -1
            for tb in range(NT):
                ps = psum_mm.tile([P, 512], f32, tag="mm")
                for h in range(H):
                    for kp in range(ND // 2):
                        nc.tensor.matmul(
                            ps[:], lhsT=uTall[:, h, 2 * kp:2 * kp + 2,
                                              tb * P:(tb + 1) * P],
                            rhs=wall[:, h, 2 * kp:2 * kp + 2, :],
                            perf_mode=DR, start=(h == 0 and kp == 0),
                            stop=False)
                nc.tensor.matmul(
                    ps[:], lhsT=ones_row[:, 0:P], rhs=brow_sb[:],
                    start=False, stop=True)
                layernorm(ps[:], OSC, resid_sb[:, tb, :], dst_sb[:, tb, :])
                if prev >= 0:
                    transpose_to(dst_sb[:, prev, :], dstT, prev)
                prev = tb
            transpose_to(dst_sb[:, prev, :], dstT, prev)

        # ============ phase 1: self attention =========================
        radd_s = x32_pool.tile([P, NS, H], f32, tag="radd_s")
        build_radd(xT8, cq_sb, tpad_sb, radd_s)

        with tc.tile_pool(name="gallp", bufs=1) as gall_pool, \
             tc.tile_pool(name="qkv", bufs=2) as qkv_pool, \
             tc.tile_pool(name="wstream", bufs=2) as wstream, \
             tc.tile_pool(name="rbc", bufs=2) as rbc_pool:
            gall = gall_pool.tile([P, H, ND, D], fp8)
            nc.gpsimd.dma_start(out=gall[:], in_=d_g8.ap().rearrange(
                "(h kt p) e -> p h kt e", p=P, h=H))

            def load_m(h):
                m_t = wstream.tile([P, ND, ND, P], fp8, tag="m")
                nc.sync.dma_start(out=m_t[:], in_=d_m8.ap()[h].rearrange(
                    "(kt p) (eb e) -> p kt eb e", p=P, e=P))
                return m_t

            def att_block(j, c, qT, radd, h):
                lo = max(j * P, c * 512)
                w = (c + 1) * 512 - lo
                ps = psum_mm.tile([P, 512], f32, tag="mm")
                for kp in range(ND // 2):
                    nc.tensor.matmul(
                        ps[:, :w],
                        lhsT=xT8[:, 2 * kp:2 * kp + 2, j * P:(j + 1) * P],
                        rhs=qT[:, 2 * kp:2 * kp + 2, lo:(c + 1) * 512],
                        perf_mode=DR, start=(kp == 0),
                        stop=(kp == ND // 2 - 1))
                if lo == j * P:
                    nc.vector.tensor_tensor(
                        out=ps[:, 0:P], in0=ps[:, 0:P], in1=diag_sb[:],
                        op=OP.add)
                nc.scalar.activation(
                    expT[:, j, lo:(c + 1) * 512], ps[:, :w], AF.Exp,
                    bias=radd[:, j, h:h + 1], scale=SCALE / WSC_M)

            m_t = load_m(0)
            qMT = qkv_pool.tile([P, ND, T], fp8, tag="qMT")
            proj_qT(m_t, xT8, qMT)
            for h in range(H):
                if h + 1 < H:
                    m_t = load_m(h + 1)
                recipT = rbc_pool.tile([1, T], f32, tag="recipT")
                recip_bc = rbc_pool.tile([P, T], f32, tag="recip_bc")
                for j in range(4):
                    att_block(j, 0, qMT, radd_s, h)
                    att_block(j, 1, qMT, radd_s, h)
                rowsum_chunk(0, 4, recipT, recip_bc)
                u_chunk(h, 0, 4, x8_sb, recip_bc)
                for j in range(4, NS):
                    att_block(j, 1, qMT, radd_s, h)
                if h + 1 < H:
                    qMT_next = qkv_pool.tile([P, ND, T], fp8, tag="qMT")
                    proj_qT(m_t, xT8, qMT_next)
                rowsum_chunk(1, NS, recipT, recip_bc)
                u_chunk(h, 1, NS, x8_sb, recip_bc)
                if h + 1 < H:
                    qMT = qMT_next

            x1_sb = x1_pool.tile([P, NT, D], f32, tag="x1")
            x1T8 = x1_pool.tile([P, ND, T], fp8, tag="x1T")
            oproj_ln(gall, bo_sb, x32_sb, x1_sb, x1T8)
        es_x32.close()

        # ============ phase 2: cross attention ========================
        with tc.tile_pool(name="mem", bufs=1) as mem_pool, \
             tc.tile_pool(name="qkv2", bufs=2) as qkv2_pool, \
             tc.tile_pool(name="wstream2", bufs=2) as wstream2, \
             tc.tile_pool(name="rbc2", bufs=2) as rbc2_pool:
            memkT8 = mem_pool.tile([P, ND, S], fp8, tag="memkT")
            nc.gpsimd.dma_start(out=memkT8[:], in_=d_memkT8.ap().rearrange(
                "(eb p) s -> p eb s", p=P))
            memv8 = mem_pool.tile([P, NS, D], fp8, tag="memv")
            nc.gpsimd.dma_start(out=memv8[:], in_=d_memv8.ap().rearrange(
                "(st p) e -> p st e", p=P))
            womall = mem_pool.tile([P, H, ND, D], fp8, tag="womall")
            nc.gpsimd.dma_start(out=womall[:], in_=d_wom8.ap().rearrange(
                "(h kt p) e -> p h kt e", p=P, h=H))
            radd_m = mem_pool.tile([P, NS, H], f32, tag="radd_m")
            build_radd(memkT8, cm_sb, spad_sb, radd_m)

            def load_wqm(h):
                wqm_t = wstream2.tile([P, ND, ND, P], fp8, tag="wqm")
                nc.sync.dma_start(out=wqm_t[:], in_=d_wqm8.ap()[h].rearrange(
                    "(kt p) (eb e) -> p kt eb e", p=P, e=P))
                return wqm_t

            def attm_block(j, c, qT, h):
                ps = psum_mm.tile([P, 512], f32, tag="mm")
                for kp in range(ND // 2):
                    nc.tensor.matmul(
                        ps[:],
                        lhsT=memkT8[:, 2 * kp:2 * kp + 2, j * P:(j + 1) * P],
                        rhs=qT[:, 2 * kp:2 * kp + 2, c * 512:(c + 1) * 512],
                        perf_mode=DR, start=(kp == 0),
                        stop=(kp == ND // 2 - 1))
                nc.scalar.activation(
                    expT[:, j, c * 512:(c + 1) * 512], ps[:], AF.Exp,
                    bias=radd_m[:, j, h:h + 1], scale=SCALE / WSC_QM)

            wqm_t = load_wqm(0)
            qmT = qkv2_pool.tile([P, ND, T], fp8, tag="qmT")
            proj_qT(wqm_t, x1T8, qmT)
            for h in range(H):
                if h + 1 < H:
                    wqm_t = load_wqm(h + 1)
                recipT = rbc2_pool.tile([1, T], f32, tag="recipT")
                recip_bc = rbc2_pool.tile([P, T], f32, tag="recip_bc")
                for c in range(NC2):
                    for j in range(NS):
                        attm_block(j, c, qmT, h)
                rowsum_chunk(0, NS, recipT, recip_bc)
                u_chunk(h, 0, NS, memv8, recip_bc)
                if h + 1 < H:
                    qmT_next = qkv2_pool.tile([P, ND, T], fp8, tag="qmT")
                    proj_qT(wqm_t, x1T8, qmT_next)
                rowsum_chunk(1, NS, recipT, recip_bc)
                u_chunk(h, 1, NS, memv8, recip_bc)
                if h + 1 < H:
                    qmT = qmT_next

            x2_sb = x2_pool.tile([P, NT, D], f32, tag="x2")
            x2T = x2_pool.tile([P, ND, T], bf16, tag="x2T")
            oproj_ln(womall, bom_sb, x1_sb, x2_sb, x2T)
        es_x1.close()
        es_attn.close()

        # ============ phase 3: FFN + LN3 (bf16) =======================
        with tc.tile_pool(name="ffn", bufs=1) as ffn_pool:
            w1_t = ffn_pool.tile([P, ND, NF, P], bf16, tag="w1")
            nc.sync.dma_start(out=w1_t[:], in_=d_w1.ap().rearrange(
                "(kt p) (fb f) -> p kt fb f", p=P, f=P))
            w2_t = ffn_pool.tile([P, NF, D], bf16, tag="w2")
            nc.sync.dma_start(out=w2_t[:], in_=d_w2.ap().rearrange(
                "(kt p) d -> p kt d", p=P))
            f1T = ffn_pool.tile([P, NF, T], bf16, tag="f1T")

            def f1_chunk(c):
                for fb in range(NF):
                    f_ps = psum_mm.tile([P, 512], f32, tag="mm")
                    for kt in range(ND):
                        nc.tensor.matmul(
                            f_ps[:], lhsT=w1_t[:, kt, fb, :],
                            rhs=x2T[:, kt, c * 512:(c + 1) * 512],
                            start=(kt == 0), stop=(kt == ND - 1))
                    nc.scalar.activation(
                        f1T[:, fb, c * 512:(c + 1) * 512], f_ps[:], AF.Relu,
                        bias=b1_sb[:, fb:fb + 1])

            f1_chunk(0)
            f1_chunk(1)
            for tb in range(NT):
                o_ps = psum_mm.tile([P, 512], f32, tag="mm")
                for kt in range(NF):
                    nc.tensor.matmul(
                        o_ps[:], lhsT=f1T[:, kt, tb * P:(tb + 1) * P],
                        rhs=w2_t[:, kt, :], start=(kt == 0), stop=False)
                nc.tensor.matmul(
                    o_ps[:], lhsT=ones_row[:, 0:P], rhs=b2_sb[:],
                    start=False, stop=True)
                out_sb = small.tile([P, D], f32, tag="out_sb")
                layernorm(o_ps[:], None, x2_sb[:, tb, :], out_sb[:])
                nc.sync.dma_start(
                    out=d_out.ap().rearrange("(tb p) d -> p tb d", p=P)[:, tb, :],
                    in_=out_sb[:])
        es_x2.close()

    nc.compile()
    _CACHE["nc"] = nc
    return nc


def make_in_maps(inputs):
    import ml_dtypes

    bf = ml_dtypes.bfloat16
    f8 = ml_dtypes.float8_e4m3
    f32 = np.float32

    wq = np.asarray(inputs["Wq_self"], f32)
    wk = np.asarray(inputs["Wk_self"], f32)
    wv = np.asarray(inputs["Wv_self"], f32)
    wo = np.asarray(inputs["Wo_self"], f32)
    bq = np.asarray(inputs["bq_self"], f32)
    bv = np.asarray(inputs["bv_self"], f32)
    wqm = np.asarray(inputs["Wq_mem"], f32)
    wom = np.asarray(inputs["Wo_mem"], f32)
    bqm = np.asarray(inputs["bq_mem"], f32)

    # fold Q/K into M = Wq Wk^T, V into G = Wv Wo_h
    m = np.matmul(wq, wk.transpose(0, 2, 1))                 # [H, D, D]
    g = np.matmul(wv, wo.reshape(H, D, D))                   # [H, D, D]
    # per-key score bias: x @ (Wk bq); the Wq bk term is softmax-invariant
    cq = np.einsum('hde,he->dh', wk, bq) * (RSC * SCALE)     # [D, H]
    cm = bqm.T * (RSC * SCALE)                               # [D, H]

    bo_row = np.asarray(inputs["bo_self"], f32).copy()
    for h in range(H):
        bo_row += bv[h] @ wo[h * D:(h + 1) * D]

    def colh(a):  # [D, H] -> [P, ND, H]
        return np.ascontiguousarray(a.reshape(ND, P, H).transpose(1, 0, 2))

    shared = {
        "m8": (m * WSC_M).astype(f8),
        "g8": (g * WSC_O).reshape(H * D, D).astype(f8),
        "cq8": colh(cq).astype(f8),
        "wqm8": (wqm * WSC_QM).astype(f8),
        "wom8": (wom * WSC_O).astype(f8),
        "cm8": colh(cm).astype(f8),
        "w1": np.asarray(inputs["W1"], f32).astype(bf),
        "w2": np.asarray(inputs["W2"], f32).astype(bf),
        "b1_c": np.ascontiguousarray(
            np.asarray(inputs["b1"], f32).reshape(NF, P).T).astype(f32),
        "bo_row": (bo_row * (WSC_O * USC)).reshape(1, D).astype(bf),
        "bom_row": (np.asarray(inputs["bo_mem"], f32)
                    * (WSC_O * USC)).reshape(1, D).astype(bf),
        "b2_row": np.asarray(inputs["b2"], f32).reshape(1, D).astype(bf),
        # attT is [s, t]: transpose the causal diagonal block
        "diag": np.ascontiguousarray(
            np.asarray(inputs["tgt_subsq_mask"], f32)[:P, :P].T),
    }
    in_maps = []
    for b in range(B):
        xb = np.asarray(inputs["x"], f32)[b]
        mk = np.asarray(inputs["mem_keys"], f32)[b]
        mvv = np.asarray(inputs["mem_values"], f32)[b]
        mm = dict(shared)
        mm["x32"] = np.ascontiguousarray(xb)
        mm["x8"] = xb.astype(f8)
        mm["xT8"] = np.ascontiguousarray(xb.T).astype(f8)
        mm["memkT8"] = np.ascontiguousarray(mk.T).astype(f8)
        mm["memv8"] = mvv.astype(f8)
        mm["tpad"] = np.ascontiguousarray(
            np.asarray(inputs["tgt_padding_mask"], f32)[b, :, 0].reshape(NS, P).T)
        mm["spad"] = np.ascontiguousarray(
            np.asarray(inputs["src_padding_mask"], f32)[b, :, 0].reshape(NS, P).T)
        in_maps.append(mm)
    return in_maps


def kernel(**inputs):
    from concourse.bass_utils import run_bass_kernel_spmd

    nc = _build()
    in_maps = make_in_maps(inputs)
    res = run_bass_kernel_spmd(nc, in_maps, list(range(B)))
    out = np.stack([np.asarray(res.results[i]["out"]) for i in range(B)])
    return out.astype(np.float32)


# revision 17
# speedup vs baseline: 2.1567x; 1.1876x over previous
"""Trainium2 Bass kernel for nn_DecoderLayer_19816979104174.

Data-parallel over batch: each of the 8 NeuronCores runs one batch element's
full decoder layer.

Key optimizations over the bf16 baseline:
  - Weight folding (host, exact in f32): M_h = Wq_h @ Wk_h^T folds the Q and
    K projections into one; G_h = Wv_h @ Wo_h folds the V projection into the
    output projection (softmax weights are applied directly to x). The
    bq-dependent score term (x @ Wk bq, per-key) folds into the exp bias; the
    bk-dependent term is softmax-invariant and dropped.
  - fp8(e4m3) DoubleRow matmuls (2x PE throughput) for all attention math.
    Weights are pre-scaled by 64 (std 0.009 -> 0.58) to clear e4m3's
    subnormal cliff; normalized attention outputs scaled by 16. All scales
    are powers of two and are undone exactly in the fp32 PSUM->SBUF flushes.
  - Output projections accumulate all heads in a single PSUM group per
    t-block (no per-head SBUF accumulation on the Vector engine).
  - x / mem_keys arrive pre-transposed/pre-quantized from the host.
  - FFN stays bf16: fp8 error there (~0.29-std signal through K=2048) would
    threaten the 2e-2 relative-error budget.
"""

import sys

sys.path.insert(0, "/opt/trn_rl_repo")
sys.path.insert(0, "/root/.axon_site/_ro/trn_rl_repo")

import numpy as np

B, T, S, D, H, F = 8, 1024, 1024, 512, 8, 2048
P = 128
NT, ND, NS, NF = T // P, D // P, S // P, F // P
NC2 = T // 512  # 512-wide t chunks
SCALE = 1.0 / float(np.sqrt(D))
LN_EPS = 1e-5

WSC_M = 64.0    # m8 = 64 * Wq@Wk^T
WSC_QM = 32.0   # wqm8 = 32 * Wq_mem  (qm values ~N(0,14.5) stay < 240)
WSC_O = 64.0    # g8 = 64 * Wv@Wo_h ; wom8 = 64 * Wo_mem
USC = 16.0      # uT = 16 * softmax-weighted sums (via ones2 = 1/16)
OSC = 1.0 / (WSC_O * USC)   # oproj PSUM flush scale = 2^-10
RSC = 1024.0    # cq8/cm8 bias-vector pre-scale
RINV = 1.0 / RSC

_CACHE = {}


def _build():
    if "nc" in _CACHE:
        return _CACHE["nc"]

    import concourse.tile as tile
    import concourse.mybir as mybir
    from concourse import bacc
    from concourse.masks import make_identity
    from contextlib import ExitStack

    bf16 = mybir.dt.bfloat16
    f32 = mybir.dt.float32
    fp8 = mybir.dt.float8e4
    AF = mybir.ActivationFunctionType
    OP = mybir.AluOpType
    DR = mybir.MatmulPerfMode.DoubleRow

    nc = bacc.Bacc("TRN2")

    # ---- DRAM I/O -----------------------------------------------------
    d_x32 = nc.dram_tensor("x32", [T, D], f32, kind="ExternalInput")
    d_x8 = nc.dram_tensor("x8", [T, D], fp8, kind="ExternalInput")
    d_xT8 = nc.dram_tensor("xT8", [D, T], fp8, kind="ExternalInput")
    d_m8 = nc.dram_tensor("m8", [H, D, D], fp8, kind="ExternalInput")
    d_g8 = nc.dram_tensor("g8", [H * D, D], fp8, kind="ExternalInput")
    d_cq8 = nc.dram_tensor("cq8", [P, ND, H], fp8, kind="ExternalInput")
    d_wqm8 = nc.dram_tensor("wqm8", [H, D, D], fp8, kind="ExternalInput")
    d_wom8 = nc.dram_tensor("wom8", [H * D, D], fp8, kind="ExternalInput")
    d_cm8 = nc.dram_tensor("cm8", [P, ND, H], fp8, kind="ExternalInput")
    d_w1 = nc.dram_tensor("w1", [D, F], bf16, kind="ExternalInput")
    d_w2 = nc.dram_tensor("w2", [F, D], bf16, kind="ExternalInput")
    d_b1 = nc.dram_tensor("b1_c", [P, NF], f32, kind="ExternalInput")
    d_bo = nc.dram_tensor("bo_row", [1, D], bf16, kind="ExternalInput")
    d_bom = nc.dram_tensor("bom_row", [1, D], bf16, kind="ExternalInput")
    d_b2 = nc.dram_tensor("b2_row", [1, D], bf16, kind="ExternalInput")
    d_memkT8 = nc.dram_tensor("memkT8", [D, S], fp8, kind="ExternalInput")
    d_memv8 = nc.dram_tensor("memv8", [S, D], fp8, kind="ExternalInput")
    d_tpad = nc.dram_tensor("tpad", [P, NS], f32, kind="ExternalInput")
    d_spad = nc.dram_tensor("spad", [P, NS], f32, kind="ExternalInput")
    d_diag = nc.dram_tensor("diag", [P, P], f32, kind="ExternalInput")
    d_out = nc.dram_tensor("out", [T, D], f32, kind="ExternalOutput")

    with tile.TileContext(nc) as tc, ExitStack() as ctx:
        const = ctx.enter_context(tc.tile_pool(name="const", bufs=1))
        small = ctx.enter_context(tc.tile_pool(name="small", bufs=2))
        psum_mm = ctx.enter_context(tc.tile_pool(name="psum_mm", bufs=4, space="PSUM"))
        psum_tr = ctx.enter_context(tc.tile_pool(name="psum_tr", bufs=2, space="PSUM"))
        psum_rs = ctx.enter_context(tc.tile_pool(name="psum_rs", bufs=2, space="PSUM"))

        # ---- constants / small inputs --------------------------------
        ident_f = const.tile([P, P], f32)
        make_identity(nc, ident_f)
        ones_sum = const.tile([P, 2, P], fp8)
        nc.vector.memset(ones_sum[:], 1.0 / USC)
        ones_row = const.tile([1, P], bf16)
        nc.vector.memset(ones_row[:], 1.0)
        eps_t = const.tile([P, 1], f32)
        nc.vector.memset(eps_t[:], LN_EPS)
        diag_sb = const.tile([P, P], f32)
        nc.gpsimd.dma_start(out=diag_sb[:], in_=d_diag.ap())
        tpad_sb = const.tile([P, NS], f32)
        nc.gpsimd.dma_start(out=tpad_sb[:], in_=d_tpad.ap())
        spad_sb = const.tile([P, NS], f32)
        nc.gpsimd.dma_start(out=spad_sb[:], in_=d_spad.ap())
        cq_sb = const.tile([P, ND, H], fp8)
        nc.gpsimd.dma_start(out=cq_sb[:], in_=d_cq8.ap())
        cm_sb = const.tile([P, ND, H], fp8)
        nc.gpsimd.dma_start(out=cm_sb[:], in_=d_cm8.ap())
        b1_sb = const.tile([P, NF], f32)
        nc.gpsimd.dma_start(out=b1_sb[:], in_=d_b1.ap())
        bo_sb = const.tile([1, D], bf16)
        nc.gpsimd.dma_start(out=bo_sb[:], in_=d_bo.ap())
        bom_sb = const.tile([1, D], bf16)
        nc.gpsimd.dma_start(out=bom_sb[:], in_=d_bom.ap())
        b2_sb = const.tile([1, D], bf16)
        nc.gpsimd.dma_start(out=b2_sb[:], in_=d_b2.ap())

        # ---- pools with phase-scoped lifetimes (LIFO close order) ----
        es_x2 = ExitStack()     # x2/x2T: FFN phase
        x2_pool = es_x2.enter_context(tc.tile_pool(name="x2p", bufs=1))
        es_attn = ExitStack()   # expT + uTall: self + cross attention
        attn_pool = es_attn.enter_context(tc.tile_pool(name="attn", bufs=1))
        es_x1 = ExitStack()     # x1/x1T: cross phase
        x1_pool = es_x1.enter_context(tc.tile_pool(name="x1p", bufs=1))
        es_x32 = ExitStack()    # x (f32/fp8/T), self radd: self phase
        x32_pool = es_x32.enter_context(tc.tile_pool(name="x32p", bufs=1))

        expT = attn_pool.tile([P, NS, T], fp8, tag="expT")
        nc.gpsimd.memset(expT[:], 0.0)
        uTall = attn_pool.tile([P, H, ND, T], fp8, tag="uTall")
        # xT8 feeds the very first PE work (radd + qMT) - DMA it first;
        # x32 is only needed at LN1, keep it off the sync queue entirely.
        xT8 = x32_pool.tile([P, ND, T], fp8, tag="xT8")
        nc.sync.dma_start(
            out=xT8[:], in_=d_xT8.ap().rearrange("(eb p) t -> p eb t", p=P))
        x8_sb = x32_pool.tile([P, NT, D], fp8, tag="x8")
        nc.sync.dma_start(
            out=x8_sb[:], in_=d_x8.ap().rearrange("(tb p) d -> p tb d", p=P))
        x32_sb = x32_pool.tile([P, NT, D], f32, tag="x32")
        nc.gpsimd.dma_start(
            out=x32_sb[:], in_=d_x32.ap().rearrange("(tb p) d -> p tb d", p=P))

        def transpose_to(src_ap, dstT, tb):
            for dt in range(ND):
                tr_ps = psum_tr.tile([P, P], f32, tag="tr")
                nc.tensor.transpose(
                    tr_ps[:], src_ap[:, dt * P:(dt + 1) * P], ident_f[:])
                nc.vector.tensor_copy(dstT[:, dt, tb * P:(tb + 1) * P], tr_ps[:])

        def layernorm(src_ap, scale, resid_ap, dst_ap):
            res = small.tile([P, D], f32, tag="ln_res")
            if scale is None:
                nc.vector.tensor_tensor(
                    out=res[:], in0=src_ap, in1=resid_ap, op=OP.add)
            else:
                nc.vector.scalar_tensor_tensor(
                    out=res[:], in0=src_ap, scalar=scale, in1=resid_ap,
                    op0=OP.mult, op1=OP.add)
            stats = small.tile([P, 6], f32, tag="ln_stats")
            nc.vector.bn_stats(stats[:], res[:])
            mv = small.tile([P, 2], f32, tag="ln_mv")
            nc.vector.bn_aggr(mv[:], stats[:])
            std = small.tile([P, 1], f32, tag="ln_std")
            nc.scalar.activation(std[:], mv[:, 1:2], AF.Sqrt, bias=eps_t[:])
            istd = small.tile([P, 1], f32, tag="ln_istd")
            nc.vector.reciprocal(istd[:], std[:])
            nc.vector.tensor_scalar(
                out=dst_ap, in0=res[:], scalar1=mv[:, 0:1], scalar2=istd[:],
                op0=OP.subtract, op1=OP.mult)

        def build_radd(lhsT_sb, c_sb, pad_sb, radd):
            """radd[:, j, h] = pad[:, j] + RINV * sum_d lhsT[d, s] c[d, h]."""
            for j in range(NS):
                r_ps = psum_mm.tile([P, 512], f32, tag="mm")
                for kp in range(ND // 2):
                    nc.tensor.matmul(
                        r_ps[:, 0:H], lhsT=lhsT_sb[:, 2 * kp:2 * kp + 2,
                                                   j * P:(j + 1) * P],
                        rhs=c_sb[:, 2 * kp:2 * kp + 2, :], perf_mode=DR,
                        start=(kp == 0), stop=(kp == ND // 2 - 1))
                nc.vector.tensor_scalar(
                    out=radd[:, j, :], in0=r_ps[:, 0:H], scalar1=RINV,
                    scalar2=pad_sb[:, j:j + 1], op0=OP.mult, op1=OP.add)

        def proj_qT(w_t, src_T8, dstT8):
            """dstT8[e, t] = sum_d w[d, e] src[d, t]   (both fp8, DoubleRow)."""
            for eb in range(ND):
                for c in range(NC2):
                    ps = psum_mm.tile([P, 512], f32, tag="mm")
                    for kp in range(ND // 2):
                        nc.tensor.matmul(
                            ps[:], lhsT=w_t[:, 2 * kp:2 * kp + 2, eb, :],
                            rhs=src_T8[:, 2 * kp:2 * kp + 2,
                                       c * 512:(c + 1) * 512],
                            perf_mode=DR, start=(kp == 0),
                            stop=(kp == ND // 2 - 1))
                    nc.scalar.activation(
                        dstT8[:, eb, c * 512:(c + 1) * 512], ps[:], AF.Copy)

        def rowsum_chunk(c, jmax, recip_bc):
            """Rowsum with a 128-wide all-ones stationary: every PSUM
            partition receives the sum, so the reciprocal runs full-width
            and no partition broadcast is needed."""
            sl = slice(c * 512, (c + 1) * 512)
            rs_ps = psum_rs.tile([P, 512], f32, tag="rs")
            for kp in range(jmax // 2):
                nc.tensor.matmul(
                    rs_ps[:], lhsT=ones_sum[:],
                    rhs=expT[:, 2 * kp:2 * kp + 2, sl], perf_mode=DR,
                    start=(kp == 0), stop=(kp == jmax // 2 - 1))
            nc.vector.reciprocal(recip_bc[:, sl], rs_ps[:])

        def u_chunk(h, c, jmax, v_sb, recip_bc):
            sl = slice(c * 512, (c + 1) * 512)
            for eb in range(ND):
                ps = psum_mm.tile([P, 512], f32, tag="mm")
                for kp in range(jmax // 2):
                    nc.tensor.matmul(
                        ps[:], lhsT=v_sb[:, 2 * kp:2 * kp + 2,
                                         eb * P:(eb + 1) * P],
                        rhs=expT[:, 2 * kp:2 * kp + 2, sl], perf_mode=DR,
                        start=(kp == 0), stop=(kp == jmax // 2 - 1))
                nc.vector.tensor_tensor(
                    out=uTall[:, h, eb, sl], in0=ps[:], in1=recip_bc[:, sl],
                    op=OP.mult)

        def oproj_ln(wall, brow_sb, resid_sb, dst_sb, dstT):
            """Per t-block: accumulate all heads' uT @ W in one PSUM group,
            add the bias row, then LN(psum*OSC + resid) -> dst (+transpose)."""
            prev = -1
            for tb in range(NT):
                ps = psum_mm.tile([P, 512], f32, tag="mm")
                for h in range(H):
                    for kp in range(ND // 2):
                        nc.tensor.matmul(
                            ps[:], lhsT=uTall[:, h, 2 * kp:2 * kp + 2,
                                              tb * P:(tb + 1) * P],
                            rhs=wall[:, h, 2 * kp:2 * kp + 2, :],
                            perf_mode=DR, start=(h == 0 and kp == 0),
                            stop=False)
                nc.tensor.matmul(
                    ps[:], lhsT=ones_row[:, 0:P], rhs=brow_sb[:],
                    start=False, stop=True)
                layernorm(ps[:], OSC, resid_sb[:, tb, :], dst_sb[:, tb, :])
                if prev >= 0:
                    transpose_to(dst_sb[:, prev, :], dstT, prev)
                prev = tb
            transpose_to(dst_sb[:, prev, :], dstT, prev)

        # ============ phase 1: self attention =========================
        radd_s = x32_pool.tile([P, NS, H], f32, tag="radd_s")
        build_radd(xT8, cq_sb, tpad_sb, radd_s)

        with tc.tile_pool(name="gallp", bufs=1) as gall_pool, \
             tc.tile_pool(name="qkv", bufs=2) as qkv_pool, \
             tc.tile_pool(name="wstream", bufs=2) as wstream, \
             tc.tile_pool(name="rbc", bufs=2) as rbc_pool:
            gall = gall_pool.tile([P, H, ND, D], fp8)
            nc.gpsimd.dma_start(out=gall[:], in_=d_g8.ap().rearrange(
                "(h kt p) e -> p h kt e", p=P, h=H))

            def load_m(h):
                m_t = wstream.tile([P, ND, ND, P], fp8, tag="m")
                nc.sync.dma_start(out=m_t[:], in_=d_m8.ap()[h].rearrange(
                    "(kt p) (eb e) -> p kt eb e", p=P, e=P))
                return m_t

            def att_block(j, c, qT, radd, h):
                lo = max(j * P, c * 512)
                w = (c + 1) * 512 - lo
                ps = psum_mm.tile([P, 512], f32, tag="mm")
                for kp in range(ND // 2):
                    nc.tensor.matmul(
                        ps[:, :w],
                        lhsT=xT8[:, 2 * kp:2 * kp + 2, j * P:(j + 1) * P],
                        rhs=qT[:, 2 * kp:2 * kp + 2, lo:(c + 1) * 512],
                        perf_mode=DR, start=(kp == 0),
                        stop=(kp == ND // 2 - 1))
                if lo == j * P:
                    nc.vector.tensor_tensor(
                        out=ps[:, 0:P], in0=ps[:, 0:P], in1=diag_sb[:],
                        op=OP.add)
                nc.scalar.activation(
                    expT[:, j, lo:(c + 1) * 512], ps[:, :w], AF.Exp,
                    bias=radd[:, j, h:h + 1], scale=SCALE / WSC_M)

            m_t = load_m(0)
            qMT = qkv_pool.tile([P, ND, T], fp8, tag="qMT")
            proj_qT(m_t, xT8, qMT)
            for h in range(H):
                if h + 1 < H:
                    m_t = load_m(h + 1)
                recip_bc = rbc_pool.tile([P, T], f32, tag="recip_bc")
                for j in range(4):
                    att_block(j, 0, qMT, radd_s, h)
                    att_block(j, 1, qMT, radd_s, h)
                rowsum_chunk(0, 4, recip_bc)
                u_chunk(h, 0, 4, x8_sb, recip_bc)
                for j in range(4, NS):
                    att_block(j, 1, qMT, radd_s, h)
                if h + 1 < H:
                    qMT_next = qkv_pool.tile([P, ND, T], fp8, tag="qMT")
                    proj_qT(m_t, xT8, qMT_next)
                rowsum_chunk(1, NS, recip_bc)
                u_chunk(h, 1, NS, x8_sb, recip_bc)
                if h + 1 < H:
                    qMT = qMT_next

            x1_sb = x1_pool.tile([P, NT, D], f32, tag="x1")
            x1T8 = x1_pool.tile([P, ND, T], fp8, tag="x1T")
            oproj_ln(gall, bo_sb, x32_sb, x1_sb, x1T8)
        es_x32.close()

        # ============ phase 2: cross attention ========================
        with tc.tile_pool(name="mem", bufs=1) as mem_pool, \
             tc.tile_pool(name="qkv2", bufs=2) as qkv2_pool, \
             tc.tile_pool(name="wstream2", bufs=2) as wstream2, \
             tc.tile_pool(name="rbc2", bufs=2) as rbc2_pool:
            memkT8 = mem_pool.tile([P, ND, S], fp8, tag="memkT")
            nc.gpsimd.dma_start(out=memkT8[:], in_=d_memkT8.ap().rearrange(
                "(eb p) s -> p eb s", p=P))
            memv8 = mem_pool.tile([P, NS, D], fp8, tag="memv")
            nc.gpsimd.dma_start(out=memv8[:], in_=d_memv8.ap().rearrange(
                "(st p) e -> p st e", p=P))
            womall = mem_pool.tile([P, H, ND, D], fp8, tag="womall")
            nc.gpsimd.dma_start(out=womall[:], in_=d_wom8.ap().rearrange(
                "(h kt p) e -> p h kt e", p=P, h=H))
            radd_m = mem_pool.tile([P, NS, H], f32, tag="radd_m")
            build_radd(memkT8, cm_sb, spad_sb, radd_m)

            def load_wqm(h):
                wqm_t = wstream2.tile([P, ND, ND, P], fp8, tag="wqm")
                nc.sync.dma_start(out=wqm_t[:], in_=d_wqm8.ap()[h].rearrange(
                    "(kt p) (eb e) -> p kt eb e", p=P, e=P))
                return wqm_t

            def attm_block(j, c, qT, h):
                ps = psum_mm.tile([P, 512], f32, tag="mm")
                for kp in range(ND // 2):
                    nc.tensor.matmul(
                        ps[:],
                        lhsT=memkT8[:, 2 * kp:2 * kp + 2, j * P:(j + 1) * P],
                        rhs=qT[:, 2 * kp:2 * kp + 2, c * 512:(c + 1) * 512],
                        perf_mode=DR, start=(kp == 0),
                        stop=(kp == ND // 2 - 1))
                nc.scalar.activation(
                    expT[:, j, c * 512:(c + 1) * 512], ps[:], AF.Exp,
                    bias=radd_m[:, j, h:h + 1], scale=SCALE / WSC_QM)

            wqm_t = load_wqm(0)
            qmT = qkv2_pool.tile([P, ND, T], fp8, tag="qmT")
            proj_qT(wqm_t, x1T8, qmT)
            for h in range(H):
                if h + 1 < H:
                    wqm_t = load_wqm(h + 1)
                recip_bc = rbc2_pool.tile([P, T], f32, tag="recip_bc")
                for c in range(NC2):
                    for j in range(NS):
                        attm_block(j, c, qmT, h)
                rowsum_chunk(0, NS, recip_bc)
                u_chunk(h, 0, NS, memv8, recip_bc)
                if h + 1 < H:
                    qmT_next = qkv2_pool.tile([P, ND, T], fp8, tag="qmT")
                    proj_qT(wqm_t, x1T8, qmT_next)
                rowsum_chunk(1, NS, recip_bc)
                u_chunk(h, 1, NS, memv8, recip_bc)
                if h + 1 < H:
                    qmT = qmT_next

            x2_sb = x2_pool.tile([P, NT, D], f32, tag="x2")
            x2T = x2_pool.tile([P, ND, T], bf16, tag="x2T")
            oproj_ln(womall, bom_sb, x1_sb, x2_sb, x2T)
        es_x1.close()
        es_attn.close()

        # ============ phase 3: FFN + LN3 (bf16) =======================
        with tc.tile_pool(name="ffn", bufs=1) as ffn_pool:
            w1_t = ffn_pool.tile([P, ND, NF, P], bf16, tag="w1")
            nc.sync.dma_start(out=w1_t[:], in_=d_w1.ap().rearrange(
                "(kt p) (fb f) -> p kt fb f", p=P, f=P))
            w2_t = ffn_pool.tile([P, NF, D], bf16, tag="w2")
            nc.sync.dma_start(out=w2_t[:], in_=d_w2.ap().rearrange(
                "(kt p) d -> p kt d", p=P))
            f1T = ffn_pool.tile([P, NF, T], bf16, tag="f1T")

            def f1_chunk(c):
                for fb in range(NF):
                    f_ps = psum_mm.tile([P, 512], f32, tag="mm")
                    for kt in range(ND):
                        nc.tensor.matmul(
                            f_ps[:], lhsT=w1_t[:, kt, fb, :],
                            rhs=x2T[:, kt, c * 512:(c + 1) * 512],
                            start=(kt == 0), stop=(kt == ND - 1))
                    nc.scalar.activation(
                        f1T[:, fb, c * 512:(c + 1) * 512], f_ps[:], AF.Relu,
                        bias=b1_sb[:, fb:fb + 1])

            f1_chunk(0)
            f1_chunk(1)
            for tb in range(NT):
                o_ps = psum_mm.tile([P, 512], f32, tag="mm")
                for kt in range(NF):
                    nc.tensor.matmul(
                        o_ps[:], lhsT=f1T[:, kt, tb * P:(tb + 1) * P],
                        rhs=w2_t[:, kt, :], start=(kt == 0), stop=False)
                nc.tensor.matmul(
                    o_ps[:], lhsT=ones_row[:, 0:P], rhs=b2_sb[:],
                    start=False, stop=True)
                out_sb = small.tile([P, D], f32, tag="out_sb")
                layernorm(o_ps[:], None, x2_sb[:, tb, :], out_sb[:])
                nc.sync.dma_start(
                    out=d_out.ap().rearrange("(tb p) d -> p tb d", p=P)[:, tb, :],
                    in_=out_sb[:])
        es_x2.close()

    nc.compile()
    _CACHE["nc"] = nc
    return nc


def make_in_maps(inputs):
    import ml_dtypes

    bf = ml_dtypes.bfloat16
    f8 = ml_dtypes.float8_e4m3
    f32 = np.float32

    wq = np.asarray(inputs["Wq_self"], f32)
    wk = np.asarray(inputs["Wk_self"], f32)
    wv = np.asarray(inputs["Wv_self"], f32)
    wo = np.asarray(inputs["Wo_self"], f32)
    bq = np.asarray(inputs["bq_self"], f32)
    bv = np.asarray(inputs["bv_self"], f32)
    wqm = np.asarray(inputs["Wq_mem"], f32)
    wom = np.asarray(inputs["Wo_mem"], f32)
    bqm = np.asarray(inputs["bq_mem"], f32)

    # fold Q/K into M = Wq Wk^T, V into G = Wv Wo_h
    m = np.matmul(wq, wk.transpose(0, 2, 1))                 # [H, D, D]
    g = np.matmul(wv, wo.reshape(H, D, D))                   # [H, D, D]
    # per-key score bias: x @ (Wk bq); the Wq bk term is softmax-invariant
    cq = np.einsum('hde,he->dh', wk, bq) * (RSC * SCALE)     # [D, H]
    cm = bqm.T * (RSC * SCALE)                               # [D, H]

    bo_row = np.asarray(inputs["bo_self"], f32).copy()
    for h in range(H):
        bo_row += bv[h] @ wo[h * D:(h + 1) * D]

    def colh(a):  # [D, H] -> [P, ND, H]
        return np.ascontiguousarray(a.reshape(ND, P, H).transpose(1, 0, 2))

    shared = {
        "m8": (m * WSC_M).astype(f8),
        "g8": (g * WSC_O).reshape(H * D, D).astype(f8),
        "cq8": colh(cq).astype(f8),
        "wqm8": (wqm * WSC_QM).astype(f8),
        "wom8": (wom * WSC_O).astype(f8),
        "cm8": colh(cm).astype(f8),
        "w1": np.asarray(inputs["W1"], f32).astype(bf),
        "w2": np.asarray(inputs["W2"], f32).astype(bf),
        "b1_c": np.ascontiguousarray(
            np.asarray(inputs["b1"], f32).reshape(NF, P).T).astype(f32),
        "bo_row": (bo_row * (WSC_O * USC)).reshape(1, D).astype(bf),
        "bom_row": (np.asarray(inputs["bo_mem"], f32)
                    * (WSC_O * USC)).reshape(1, D).astype(bf),
        "b2_row": np.asarray(inputs["b2"], f32).reshape(1, D).astype(bf),
        # attT is [s, t]: transpose the causal diagonal block
        "diag": np.ascontiguousarray(
            np.asarray(inputs["tgt_subsq_mask"], f32)[:P, :P].T),
    }
    in_maps = []
    for b in range(B):
        xb = np.asarray(inputs["x"], f32)[b]
        mk = np.asarray(inputs["mem_keys"], f32)[b]
        mvv = np.asarray(inputs["mem_values"], f32)[b]
        mm = dict(shared)
        mm["x32"] = np.ascontiguousarray(xb)
        mm["x8"] = xb.astype(f8)
        mm["xT8"] = np.ascontiguousarray(xb.T).astype(f8)
        mm["memkT8"] = np.ascontiguousarray(mk.T).astype(f8)
        mm["memv8"] = mvv.astype(f8)
        mm["tpad"] = np.ascontiguousarray(
            np.asarray(inputs["tgt_padding_mask"], f32)[b, :, 0].reshape(NS, P).T)
        mm["spad"] = np.ascontiguousarray(
            np.asarray(inputs["src_padding_mask"], f32)[b, :, 0].reshape(NS, P).T)
        in_maps.append(mm)
    return in_maps


def kernel(**inputs):
    from concourse.bass_utils import run_bass_kernel_spmd

    nc = _build()
    in_maps = make_in_maps(inputs)
    res = run_bass_kernel_spmd(nc, in_maps, list(range(B)))
    out = np.stack([np.asarray(res.results[i]["out"]) for i in range(B)])
    return out.astype(np.float32)


# revision 18
# speedup vs baseline: 2.1750x; 1.0085x over previous
"""Trainium2 Bass kernel for nn_DecoderLayer_19816979104174.

Data-parallel over batch: each of the 8 NeuronCores runs one batch element's
full decoder layer.

Key optimizations over the bf16 baseline:
  - Weight folding (host, exact in f32): M_h = Wq_h @ Wk_h^T folds the Q and
    K projections into one; G_h = Wv_h @ Wo_h folds the V projection into the
    output projection (softmax weights are applied directly to x). The
    bq-dependent score term (x @ Wk bq, per-key) folds into the exp bias; the
    bk-dependent term is softmax-invariant and dropped.
  - fp8(e4m3) DoubleRow matmuls (2x PE throughput) for all attention math.
    Weights are pre-scaled by 64 (std 0.009 -> 0.58) to clear e4m3's
    subnormal cliff; normalized attention outputs scaled by 16. All scales
    are powers of two and are undone exactly in the fp32 PSUM->SBUF flushes.
  - Output projections accumulate all heads in a single PSUM group per
    t-block (no per-head SBUF accumulation on the Vector engine).
  - x / mem_keys arrive pre-transposed/pre-quantized from the host.
  - FFN stays bf16: fp8 error there (~0.29-std signal through K=2048) would
    threaten the 2e-2 relative-error budget.
"""

import sys

sys.path.insert(0, "/opt/trn_rl_repo")
sys.path.insert(0, "/root/.axon_site/_ro/trn_rl_repo")

import numpy as np

B, T, S, D, H, F = 8, 1024, 1024, 512, 8, 2048
P = 128
NT, ND, NS, NF = T // P, D // P, S // P, F // P
NC2 = T // 512  # 512-wide t chunks
SCALE = 1.0 / float(np.sqrt(D))
LN_EPS = 1e-5

WSC_M = 64.0    # m8 = 64 * Wq@Wk^T
WSC_QM = 32.0   # wqm8 = 32 * Wq_mem  (qm values ~N(0,14.5) stay < 240)
WSC_O = 64.0    # g8 = 64 * Wv@Wo_h ; wom8 = 64 * Wo_mem
USC = 16.0      # uT = 16 * softmax-weighted sums (via ones2 = 1/16)
OSC = 1.0 / (WSC_O * USC)   # oproj PSUM flush scale = 2^-10
RSC = 1024.0    # cq8/cm8 bias-vector pre-scale
RINV = 1.0 / RSC

_CACHE = {}


def _build():
    if "nc" in _CACHE:
        return _CACHE["nc"]

    import concourse.tile as tile
    import concourse.mybir as mybir
    from concourse import bacc
    from concourse.masks import make_identity
    from contextlib import ExitStack

    bf16 = mybir.dt.bfloat16
    f32 = mybir.dt.float32
    fp8 = mybir.dt.float8e4
    AF = mybir.ActivationFunctionType
    OP = mybir.AluOpType
    DR = mybir.MatmulPerfMode.DoubleRow

    nc = bacc.Bacc("TRN2")

    # ---- DRAM I/O -----------------------------------------------------
    d_x32 = nc.dram_tensor("x32", [T, D], f32, kind="ExternalInput")
    d_x8 = nc.dram_tensor("x8", [T, D], fp8, kind="ExternalInput")
    d_xT8 = nc.dram_tensor("xT8", [D, T], fp8, kind="ExternalInput")
    d_m8 = nc.dram_tensor("m8", [H, D, D], fp8, kind="ExternalInput")
    d_g8 = nc.dram_tensor("g8", [H * D, D], fp8, kind="ExternalInput")
    d_cq8 = nc.dram_tensor("cq8", [P, ND, H], fp8, kind="ExternalInput")
    d_wqm8 = nc.dram_tensor("wqm8", [H, D, D], fp8, kind="ExternalInput")
    d_wom8 = nc.dram_tensor("wom8", [H * D, D], fp8, kind="ExternalInput")
    d_cm8 = nc.dram_tensor("cm8", [P, ND, H], fp8, kind="ExternalInput")
    d_w1 = nc.dram_tensor("w1", [D, F], bf16, kind="ExternalInput")
    d_w2 = nc.dram_tensor("w2", [F, D], bf16, kind="ExternalInput")
    d_b1 = nc.dram_tensor("b1_c", [P, NF], f32, kind="ExternalInput")
    d_bo = nc.dram_tensor("bo_row", [1, D], bf16, kind="ExternalInput")
    d_bom = nc.dram_tensor("bom_row", [1, D], bf16, kind="ExternalInput")
    d_b2 = nc.dram_tensor("b2_row", [1, D], bf16, kind="ExternalInput")
    d_memkT8 = nc.dram_tensor("memkT8", [D, S], fp8, kind="ExternalInput")
    d_memv8 = nc.dram_tensor("memv8", [S, D], fp8, kind="ExternalInput")
    d_tpad = nc.dram_tensor("tpad", [P, NS], f32, kind="ExternalInput")
    d_spad = nc.dram_tensor("spad", [P, NS], f32, kind="ExternalInput")
    d_diag = nc.dram_tensor("diag", [P, P], f32, kind="ExternalInput")
    d_out = nc.dram_tensor("out", [T, D], f32, kind="ExternalOutput")

    with tile.TileContext(nc) as tc, ExitStack() as ctx:
        const = ctx.enter_context(tc.tile_pool(name="const", bufs=1))
        small = ctx.enter_context(tc.tile_pool(name="small", bufs=2))
        psum_mm = ctx.enter_context(tc.tile_pool(name="psum_mm", bufs=5, space="PSUM"))
        psum_tr = ctx.enter_context(tc.tile_pool(name="psum_tr", bufs=1, space="PSUM"))
        psum_rs = ctx.enter_context(tc.tile_pool(name="psum_rs", bufs=2, space="PSUM"))

        # ---- constants / small inputs --------------------------------
        ident_f = const.tile([P, P], f32)
        make_identity(nc, ident_f)
        ones_sum = const.tile([P, 2, P], fp8)
        nc.vector.memset(ones_sum[:], 1.0 / USC)
        ones_row = const.tile([1, P], bf16)
        nc.vector.memset(ones_row[:], 1.0)
        eps_t = const.tile([P, 1], f32)
        nc.vector.memset(eps_t[:], LN_EPS)
        diag_sb = const.tile([P, P], f32)
        nc.gpsimd.dma_start(out=diag_sb[:], in_=d_diag.ap())
        tpad_sb = const.tile([P, NS], f32)
        nc.gpsimd.dma_start(out=tpad_sb[:], in_=d_tpad.ap())
        spad_sb = const.tile([P, NS], f32)
        nc.gpsimd.dma_start(out=spad_sb[:], in_=d_spad.ap())
        cq_sb = const.tile([P, ND, H], fp8)
        nc.gpsimd.dma_start(out=cq_sb[:], in_=d_cq8.ap())
        cm_sb = const.tile([P, ND, H], fp8)
        nc.gpsimd.dma_start(out=cm_sb[:], in_=d_cm8.ap())
        b1_sb = const.tile([P, NF], f32)
        nc.gpsimd.dma_start(out=b1_sb[:], in_=d_b1.ap())
        bo_sb = const.tile([1, D], bf16)
        nc.gpsimd.dma_start(out=bo_sb[:], in_=d_bo.ap())
        bom_sb = const.tile([1, D], bf16)
        nc.gpsimd.dma_start(out=bom_sb[:], in_=d_bom.ap())
        b2_sb = const.tile([1, D], bf16)
        nc.gpsimd.dma_start(out=b2_sb[:], in_=d_b2.ap())

        # ---- pools with phase-scoped lifetimes (LIFO close order) ----
        es_x2 = ExitStack()     # x2/x2T: FFN phase
        x2_pool = es_x2.enter_context(tc.tile_pool(name="x2p", bufs=1))
        es_attn = ExitStack()   # expT + uTall: self + cross attention
        attn_pool = es_attn.enter_context(tc.tile_pool(name="attn", bufs=1))
        es_x1 = ExitStack()     # x1/x1T: cross phase
        x1_pool = es_x1.enter_context(tc.tile_pool(name="x1p", bufs=1))
        es_x32 = ExitStack()    # x (f32/fp8/T), self radd: self phase
        x32_pool = es_x32.enter_context(tc.tile_pool(name="x32p", bufs=1))

        expT = attn_pool.tile([P, NS, T], fp8, tag="expT")
        nc.gpsimd.memset(expT[:], 0.0)
        uTall = attn_pool.tile([P, H, ND, T], fp8, tag="uTall")
        # xT8 feeds the very first PE work (radd + qMT) - DMA it first;
        # x32 is only needed at LN1, keep it off the sync queue entirely.
        xT8 = x32_pool.tile([P, ND, T], fp8, tag="xT8")
        nc.sync.dma_start(
            out=xT8[:], in_=d_xT8.ap().rearrange("(eb p) t -> p eb t", p=P))
        x8_sb = x32_pool.tile([P, NT, D], fp8, tag="x8")
        nc.sync.dma_start(
            out=x8_sb[:], in_=d_x8.ap().rearrange("(tb p) d -> p tb d", p=P))
        x32_sb = x32_pool.tile([P, NT, D], f32, tag="x32")
        nc.gpsimd.dma_start(
            out=x32_sb[:], in_=d_x32.ap().rearrange("(tb p) d -> p tb d", p=P))

        def transpose_to(src_ap, dstT, tb):
            for dt in range(ND):
                tr_ps = psum_tr.tile([P, P], f32, tag="tr")
                nc.tensor.transpose(
                    tr_ps[:], src_ap[:, dt * P:(dt + 1) * P], ident_f[:])
                nc.vector.tensor_copy(dstT[:, dt, tb * P:(tb + 1) * P], tr_ps[:])

        def layernorm(src_ap, scale, resid_ap, dst_ap):
            res = small.tile([P, D], f32, tag="ln_res")
            if scale is None:
                nc.vector.tensor_tensor(
                    out=res[:], in0=src_ap, in1=resid_ap, op=OP.add)
            else:
                nc.vector.scalar_tensor_tensor(
                    out=res[:], in0=src_ap, scalar=scale, in1=resid_ap,
                    op0=OP.mult, op1=OP.add)
            stats = small.tile([P, 6], f32, tag="ln_stats")
            nc.vector.bn_stats(stats[:], res[:])
            mv = small.tile([P, 2], f32, tag="ln_mv")
            nc.vector.bn_aggr(mv[:], stats[:])
            std = small.tile([P, 1], f32, tag="ln_std")
            nc.scalar.activation(std[:], mv[:, 1:2], AF.Sqrt, bias=eps_t[:])
            istd = small.tile([P, 1], f32, tag="ln_istd")
            nc.vector.reciprocal(istd[:], std[:])
            nc.vector.tensor_scalar(
                out=dst_ap, in0=res[:], scalar1=mv[:, 0:1], scalar2=istd[:],
                op0=OP.subtract, op1=OP.mult)

        def build_radd(lhsT_sb, c_sb, pad_sb, radd):
            """radd[:, j, h] = pad[:, j] + RINV * sum_d lhsT[d, s] c[d, h]."""
            for j in range(NS):
                r_ps = psum_mm.tile([P, 512], f32, tag="mm")
                for kp in range(ND // 2):
                    nc.tensor.matmul(
                        r_ps[:, 0:H], lhsT=lhsT_sb[:, 2 * kp:2 * kp + 2,
                                                   j * P:(j + 1) * P],
                        rhs=c_sb[:, 2 * kp:2 * kp + 2, :], perf_mode=DR,
                        start=(kp == 0), stop=(kp == ND // 2 - 1))
                nc.vector.tensor_scalar(
                    out=radd[:, j, :], in0=r_ps[:, 0:H], scalar1=RINV,
                    scalar2=pad_sb[:, j:j + 1], op0=OP.mult, op1=OP.add)

        def proj_qT(w_t, src_T8, dstT8):
            """dstT8[e, t] = sum_d w[d, e] src[d, t]   (both fp8, DoubleRow)."""
            for eb in range(ND):
                for c in range(NC2):
                    ps = psum_mm.tile([P, 512], f32, tag="mm")
                    for kp in range(ND // 2):
                        nc.tensor.matmul(
                            ps[:], lhsT=w_t[:, 2 * kp:2 * kp + 2, eb, :],
                            rhs=src_T8[:, 2 * kp:2 * kp + 2,
                                       c * 512:(c + 1) * 512],
                            perf_mode=DR, start=(kp == 0),
                            stop=(kp == ND // 2 - 1))
                    nc.scalar.activation(
                        dstT8[:, eb, c * 512:(c + 1) * 512], ps[:], AF.Copy)

        def rowsum_chunk(c, jmax, recip_bc):
            """Rowsum with a 128-wide all-ones stationary: every PSUM
            partition receives the sum, so the reciprocal runs full-width
            and no partition broadcast is needed."""
            sl = slice(c * 512, (c + 1) * 512)
            rs_ps = psum_rs.tile([P, 512], f32, tag="rs")
            for kp in range(jmax // 2):
                nc.tensor.matmul(
                    rs_ps[:], lhsT=ones_sum[:],
                    rhs=expT[:, 2 * kp:2 * kp + 2, sl], perf_mode=DR,
                    start=(kp == 0), stop=(kp == jmax // 2 - 1))
            nc.vector.reciprocal(recip_bc[:, sl], rs_ps[:])

        def u_chunk(h, c, jmax, v_sb, recip_bc):
            sl = slice(c * 512, (c + 1) * 512)
            for eb in range(ND):
                ps = psum_mm.tile([P, 512], f32, tag="mm")
                for kp in range(jmax // 2):
                    nc.tensor.matmul(
                        ps[:], lhsT=v_sb[:, 2 * kp:2 * kp + 2,
                                         eb * P:(eb + 1) * P],
                        rhs=expT[:, 2 * kp:2 * kp + 2, sl], perf_mode=DR,
                        start=(kp == 0), stop=(kp == jmax // 2 - 1))
                nc.vector.tensor_tensor(
                    out=uTall[:, h, eb, sl], in0=ps[:], in1=recip_bc[:, sl],
                    op=OP.mult)

        def oproj_ln(wall, brow_sb, resid_sb, dst_sb, dstT):
            """Per t-block: accumulate all heads' uT @ W in one PSUM group,
            add the bias row, then LN(psum*OSC + resid) -> dst (+transpose)."""
            prev = -1
            for tb in range(NT):
                ps = psum_mm.tile([P, 512], f32, tag="mm")
                for h in range(H):
                    for kp in range(ND // 2):
                        nc.tensor.matmul(
                            ps[:], lhsT=uTall[:, h, 2 * kp:2 * kp + 2,
                                              tb * P:(tb + 1) * P],
                            rhs=wall[:, h, 2 * kp:2 * kp + 2, :],
                            perf_mode=DR, start=(h == 0 and kp == 0),
                            stop=False)
                nc.tensor.matmul(
                    ps[:], lhsT=ones_row[:, 0:P], rhs=brow_sb[:],
                    start=False, stop=True)
                layernorm(ps[:], OSC, resid_sb[:, tb, :], dst_sb[:, tb, :])
                if prev >= 0:
                    transpose_to(dst_sb[:, prev, :], dstT, prev)
                prev = tb
            transpose_to(dst_sb[:, prev, :], dstT, prev)

        # ============ phase 1: self attention =========================
        radd_s = x32_pool.tile([P, NS, H], f32, tag="radd_s")
        build_radd(xT8, cq_sb, tpad_sb, radd_s)

        with tc.tile_pool(name="gallp", bufs=1) as gall_pool, \
             tc.tile_pool(name="qkv", bufs=2) as qkv_pool, \
             tc.tile_pool(name="wstream", bufs=2) as wstream, \
             tc.tile_pool(name="rbc", bufs=2) as rbc_pool:
            gall = gall_pool.tile([P, H, ND, D], fp8)
            nc.gpsimd.dma_start(out=gall[:], in_=d_g8.ap().rearrange(
                "(h kt p) e -> p h kt e", p=P, h=H))

            def load_m(h):
                m_t = wstream.tile([P, ND, ND, P], fp8, tag="m")
                nc.sync.dma_start(out=m_t[:], in_=d_m8.ap()[h].rearrange(
                    "(kt p) (eb e) -> p kt eb e", p=P, e=P))
                return m_t

            def att_block(j, c, qT, radd, h):
                lo = max(j * P, c * 512)
                w = (c + 1) * 512 - lo
                ps = psum_mm.tile([P, 512], f32, tag="mm")
                for kp in range(ND // 2):
                    nc.tensor.matmul(
                        ps[:, :w],
                        lhsT=xT8[:, 2 * kp:2 * kp + 2, j * P:(j + 1) * P],
                        rhs=qT[:, 2 * kp:2 * kp + 2, lo:(c + 1) * 512],
                        perf_mode=DR, start=(kp == 0),
                        stop=(kp == ND // 2 - 1))
                if lo == j * P:
                    nc.vector.tensor_tensor(
                        out=ps[:, 0:P], in0=ps[:, 0:P], in1=diag_sb[:],
                        op=OP.add)
                nc.scalar.activation(
                    expT[:, j, lo:(c + 1) * 512], ps[:, :w], AF.Exp,
                    bias=radd[:, j, h:h + 1], scale=SCALE / WSC_M)

            m_t = load_m(0)
            qMT = qkv_pool.tile([P, ND, T], fp8, tag="qMT")
            proj_qT(m_t, xT8, qMT)
            for h in range(H):
                if h + 1 < H:
                    m_t = load_m(h + 1)
                recip_bc = rbc_pool.tile([P, T], f32, tag="recip_bc")
                for j in range(4):
                    att_block(j, 0, qMT, radd_s, h)
                    att_block(j, 1, qMT, radd_s, h)
                rowsum_chunk(0, 4, recip_bc)
                u_chunk(h, 0, 4, x8_sb, recip_bc)
                for j in range(4, NS):
                    att_block(j, 1, qMT, radd_s, h)
                if h + 1 < H:
                    qMT_next = qkv_pool.tile([P, ND, T], fp8, tag="qMT")
                    proj_qT(m_t, xT8, qMT_next)
                rowsum_chunk(1, NS, recip_bc)
                u_chunk(h, 1, NS, x8_sb, recip_bc)
                if h + 1 < H:
                    qMT = qMT_next

            x1_sb = x1_pool.tile([P, NT, D], f32, tag="x1")
            x1T8 = x1_pool.tile([P, ND, T], fp8, tag="x1T")
            oproj_ln(gall, bo_sb, x32_sb, x1_sb, x1T8)
        es_x32.close()

        # ============ phase 2: cross attention ========================
        with tc.tile_pool(name="mem", bufs=1) as mem_pool, \
             tc.tile_pool(name="qkv2", bufs=2) as qkv2_pool, \
             tc.tile_pool(name="wstream2", bufs=2) as wstream2, \
             tc.tile_pool(name="rbc2", bufs=2) as rbc2_pool:
            memkT8 = mem_pool.tile([P, ND, S], fp8, tag="memkT")
            nc.gpsimd.dma_start(out=memkT8[:], in_=d_memkT8.ap().rearrange(
                "(eb p) s -> p eb s", p=P))
            memv8 = mem_pool.tile([P, NS, D], fp8, tag="memv")
            nc.gpsimd.dma_start(out=memv8[:], in_=d_memv8.ap().rearrange(
                "(st p) e -> p st e", p=P))
            womall = mem_pool.tile([P, H, ND, D], fp8, tag="womall")
            nc.gpsimd.dma_start(out=womall[:], in_=d_wom8.ap().rearrange(
                "(h kt p) e -> p h kt e", p=P, h=H))
            radd_m = mem_pool.tile([P, NS, H], f32, tag="radd_m")
            build_radd(memkT8, cm_sb, spad_sb, radd_m)

            def load_wqm(h):
                wqm_t = wstream2.tile([P, ND, ND, P], fp8, tag="wqm")
                nc.sync.dma_start(out=wqm_t[:], in_=d_wqm8.ap()[h].rearrange(
                    "(kt p) (eb e) -> p kt eb e", p=P, e=P))
                return wqm_t

            def attm_block(j, c, qT, h):
                ps = psum_mm.tile([P, 512], f32, tag="mm")
                for kp in range(ND // 2):
                    nc.tensor.matmul(
                        ps[:],
                        lhsT=memkT8[:, 2 * kp:2 * kp + 2, j * P:(j + 1) * P],
                        rhs=qT[:, 2 * kp:2 * kp + 2, c * 512:(c + 1) * 512],
                        perf_mode=DR, start=(kp == 0),
                        stop=(kp == ND // 2 - 1))
                nc.scalar.activation(
                    expT[:, j, c * 512:(c + 1) * 512], ps[:], AF.Exp,
                    bias=radd_m[:, j, h:h + 1], scale=SCALE / WSC_QM)

            wqm_t = load_wqm(0)
            qmT = qkv2_pool.tile([P, ND, T], fp8, tag="qmT")
            proj_qT(wqm_t, x1T8, qmT)
            for h in range(H):
                if h + 1 < H:
                    wqm_t = load_wqm(h + 1)
                recip_bc = rbc2_pool.tile([P, T], f32, tag="recip_bc")
                for c in range(NC2):
                    for j in range(NS):
                        attm_block(j, c, qmT, h)
                rowsum_chunk(0, NS, recip_bc)
                u_chunk(h, 0, NS, memv8, recip_bc)
                if h + 1 < H:
                    qmT_next = qkv2_pool.tile([P, ND, T], fp8, tag="qmT")
                    proj_qT(wqm_t, x1T8, qmT_next)
                rowsum_chunk(1, NS, recip_bc)
                u_chunk(h, 1, NS, memv8, recip_bc)
                if h + 1 < H:
                    qmT = qmT_next

            x2_sb = x2_pool.tile([P, NT, D], f32, tag="x2")
            x2T = x2_pool.tile([P, ND, T], bf16, tag="x2T")
            oproj_ln(womall, bom_sb, x1_sb, x2_sb, x2T)
        es_x1.close()
        es_attn.close()

        # ============ phase 3: FFN + LN3 (bf16) =======================
        with tc.tile_pool(name="ffn", bufs=1) as ffn_pool:
            w1_t = ffn_pool.tile([P, ND, NF, P], bf16, tag="w1")
            nc.sync.dma_start(out=w1_t[:], in_=d_w1.ap().rearrange(
                "(kt p) (fb f) -> p kt fb f", p=P, f=P))
            w2_t = ffn_pool.tile([P, NF, D], bf16, tag="w2")
            nc.sync.dma_start(out=w2_t[:], in_=d_w2.ap().rearrange(
                "(kt p) d -> p kt d", p=P))
            f1T = ffn_pool.tile([P, NF, T], bf16, tag="f1T")

            def f1_chunk(c):
                for fb in range(NF):
                    f_ps = psum_mm.tile([P, 512], f32, tag="mm")
                    for kt in range(ND):
                        nc.tensor.matmul(
                            f_ps[:], lhsT=w1_t[:, kt, fb, :],
                            rhs=x2T[:, kt, c * 512:(c + 1) * 512],
                            start=(kt == 0), stop=(kt == ND - 1))
                    nc.scalar.activation(
                        f1T[:, fb, c * 512:(c + 1) * 512], f_ps[:], AF.Relu,
                        bias=b1_sb[:, fb:fb + 1])

            f1_chunk(0)
            f1_chunk(1)
            for tb in range(NT):
                o_ps = psum_mm.tile([P, 512], f32, tag="mm")
                for kt in range(NF):
                    nc.tensor.matmul(
                        o_ps[:], lhsT=f1T[:, kt, tb * P:(tb + 1) * P],
                        rhs=w2_t[:, kt, :], start=(kt == 0), stop=False)
                nc.tensor.matmul(
                    o_ps[:], lhsT=ones_row[:, 0:P], rhs=b2_sb[:],
                    start=False, stop=True)
                out_sb = small.tile([P, D], f32, tag="out_sb")
                layernorm(o_ps[:], None, x2_sb[:, tb, :], out_sb[:])
                nc.sync.dma_start(
                    out=d_out.ap().rearrange("(tb p) d -> p tb d", p=P)[:, tb, :],
                    in_=out_sb[:])
        es_x2.close()

    nc.compile()
    _CACHE["nc"] = nc
    return nc


def make_in_maps(inputs):
    import ml_dtypes

    bf = ml_dtypes.bfloat16
    f8 = ml_dtypes.float8_e4m3
    f32 = np.float32

    wq = np.asarray(inputs["Wq_self"], f32)
    wk = np.asarray(inputs["Wk_self"], f32)
    wv = np.asarray(inputs["Wv_self"], f32)
    wo = np.asarray(inputs["Wo_self"], f32)
    bq = np.asarray(inputs["bq_self"], f32)
    bv = np.asarray(inputs["bv_self"], f32)
    wqm = np.asarray(inputs["Wq_mem"], f32)
    wom = np.asarray(inputs["Wo_mem"], f32)
    bqm = np.asarray(inputs["bq_mem"], f32)

    # fold Q/K into M = Wq Wk^T, V into G = Wv Wo_h
    m = np.matmul(wq, wk.transpose(0, 2, 1))                 # [H, D, D]
    g = np.matmul(wv, wo.reshape(H, D, D))                   # [H, D, D]
    # per-key score bias: x @ (Wk bq); the Wq bk term is softmax-invariant
    cq = np.einsum('hde,he->dh', wk, bq) * (RSC * SCALE)     # [D, H]
    cm = bqm.T * (RSC * SCALE)                               # [D, H]

    bo_row = np.asarray(inputs["bo_self"], f32).copy()
    for h in range(H):
        bo_row += bv[h] @ wo[h * D:(h + 1) * D]

    def colh(a):  # [D, H] -> [P, ND, H]
        return np.ascontiguousarray(a.reshape(ND, P, H).transpose(1, 0, 2))

    shared = {
        "m8": (m * WSC_M).astype(f8),
        "g8": (g * WSC_O).reshape(H * D, D).astype(f8),
        "cq8": colh(cq).astype(f8),
        "wqm8": (wqm * WSC_QM).astype(f8),
        "wom8": (wom * WSC_O).astype(f8),
        "cm8": colh(cm).astype(f8),
        "w1": np.asarray(inputs["W1"], f32).astype(bf),
        "w2": np.asarray(inputs["W2"], f32).astype(bf),
        "b1_c": np.ascontiguousarray(
            np.asarray(inputs["b1"], f32).reshape(NF, P).T).astype(f32),
        "bo_row": (bo_row * (WSC_O * USC)).reshape(1, D).astype(bf),
        "bom_row": (np.asarray(inputs["bo_mem"], f32)
                    * (WSC_O * USC)).reshape(1, D).astype(bf),
        "b2_row": np.asarray(inputs["b2"], f32).reshape(1, D).astype(bf),
        # attT is [s, t]: transpose the causal diagonal block
        "diag": np.ascontiguousarray(
            np.asarray(inputs["tgt_subsq_mask"], f32)[:P, :P].T),
    }
    in_maps = []
    for b in range(B):
        xb = np.asarray(inputs["x"], f32)[b]
        mk = np.asarray(inputs["mem_keys"], f32)[b]
        mvv = np.asarray(inputs["mem_values"], f32)[b]
        mm = dict(shared)
        mm["x32"] = np.ascontiguousarray(xb)
        mm["x8"] = xb.astype(f8)
        mm["xT8"] = np.ascontiguousarray(xb.T).astype(f8)
        mm["memkT8"] = np.ascontiguousarray(mk.T).astype(f8)
        mm["memv8"] = mvv.astype(f8)
        mm["tpad"] = np.ascontiguousarray(
            np.asarray(inputs["tgt_padding_mask"], f32)[b, :, 0].reshape(NS, P).T)
        mm["spad"] = np.ascontiguousarray(
            np.asarray(inputs["src_padding_mask"], f32)[b, :, 0].reshape(NS, P).T)
        in_maps.append(mm)
    return in_maps


def kernel(**inputs):
    from concourse.bass_utils import run_bass_kernel_spmd

    nc = _build()
    in_maps = make_in_maps(inputs)
    res = run_bass_kernel_spmd(nc, in_maps, list(range(B)))
    out = np.stack([np.asarray(res.results[i]["out"]) for i in range(B)])
    return out.astype(np.float32)


# revision 21
# speedup vs baseline: 2.2198x; 1.0206x over previous
"""Trainium2 Bass kernel for nn_DecoderLayer_19816979104174.

Data-parallel over batch: each of the 8 NeuronCores runs one batch element's
full decoder layer.

Key optimizations over the bf16 baseline:
  - Weight folding (host, exact in f32): M_h = Wq_h @ Wk_h^T folds the Q and
    K projections into one; G_h = Wv_h @ Wo_h folds the V projection into the
    output projection (softmax weights are applied directly to x). The
    bq-dependent score term (x @ Wk bq, per-key) folds into the exp bias; the
    bk-dependent term is softmax-invariant and dropped.
  - fp8(e4m3) DoubleRow matmuls (2x PE throughput) for all attention math.
    Weights are pre-scaled by 64 (std 0.009 -> 0.58) to clear e4m3's
    subnormal cliff; normalized attention outputs scaled by 16. All scales
    are powers of two and are undone exactly in the fp32 PSUM->SBUF flushes.
  - Output projections accumulate all heads in a single PSUM group per
    t-block (no per-head SBUF accumulation on the Vector engine).
  - x / mem_keys arrive pre-transposed/pre-quantized from the host.
  - FFN stays bf16: fp8 error there (~0.29-std signal through K=2048) would
    threaten the 2e-2 relative-error budget.
"""

import sys

sys.path.insert(0, "/opt/trn_rl_repo")
sys.path.insert(0, "/root/.axon_site/_ro/trn_rl_repo")

import numpy as np

B, T, S, D, H, F = 8, 1024, 1024, 512, 8, 2048
P = 128
NT, ND, NS, NF = T // P, D // P, S // P, F // P
NC2 = T // 512  # 512-wide t chunks
SCALE = 1.0 / float(np.sqrt(D))
LN_EPS = 1e-5

WSC_M = 64.0    # m8 = 64 * Wq@Wk^T
WSC_QM = 32.0   # wqm8 = 32 * Wq_mem  (qm values ~N(0,14.5) stay < 240)
WSC_O = 64.0    # g8 = 64 * Wv@Wo_h ; wom8 = 64 * Wo_mem
USC = 16.0      # uT = 16 * softmax-weighted sums (via ones2 = 1/16)
OSC = 1.0 / (WSC_O * USC)   # oproj PSUM flush scale = 2^-10
RSC = 1024.0    # cq8/cm8 bias-vector pre-scale
RINV = 1.0 / RSC

_CACHE = {}


def _build():
    if "nc" in _CACHE:
        return _CACHE["nc"]

    import concourse.tile as tile
    import concourse.mybir as mybir
    from concourse import bacc
    from concourse.masks import make_identity
    from contextlib import ExitStack

    bf16 = mybir.dt.bfloat16
    f32 = mybir.dt.float32
    fp8 = mybir.dt.float8e4
    AF = mybir.ActivationFunctionType
    OP = mybir.AluOpType
    DR = mybir.MatmulPerfMode.DoubleRow

    nc = bacc.Bacc("TRN2")

    # ---- DRAM I/O -----------------------------------------------------
    d_x32 = nc.dram_tensor("x32", [T, D], f32, kind="ExternalInput")
    d_x8 = nc.dram_tensor("x8", [T, D], fp8, kind="ExternalInput")
    d_xT8 = nc.dram_tensor("xT8", [D, T], fp8, kind="ExternalInput")
    d_m8 = nc.dram_tensor("m8", [H, D, D], fp8, kind="ExternalInput")
    d_g8 = nc.dram_tensor("g8", [H * D, D], fp8, kind="ExternalInput")
    d_cq8 = nc.dram_tensor("cq8", [P, ND, H], fp8, kind="ExternalInput")
    d_wqm8 = nc.dram_tensor("wqm8", [H, D, D], fp8, kind="ExternalInput")
    d_wom8 = nc.dram_tensor("wom8", [H * D, D], fp8, kind="ExternalInput")
    d_cm8 = nc.dram_tensor("cm8", [P, ND, H], fp8, kind="ExternalInput")
    d_w1 = nc.dram_tensor("w1", [D, F], bf16, kind="ExternalInput")
    d_w2 = nc.dram_tensor("w2", [F, D], bf16, kind="ExternalInput")
    d_b1 = nc.dram_tensor("b1_c", [P, NF], f32, kind="ExternalInput")
    d_bo = nc.dram_tensor("bo_row", [1, D], bf16, kind="ExternalInput")
    d_bom = nc.dram_tensor("bom_row", [1, D], bf16, kind="ExternalInput")
    d_b2 = nc.dram_tensor("b2_row", [1, D], bf16, kind="ExternalInput")
    d_memkT8 = nc.dram_tensor("memkT8", [D, S], fp8, kind="ExternalInput")
    d_memv8 = nc.dram_tensor("memv8", [S, D], fp8, kind="ExternalInput")
    d_tpad = nc.dram_tensor("tpad", [P, NS], f32, kind="ExternalInput")
    d_spad = nc.dram_tensor("spad", [P, NS], f32, kind="ExternalInput")
    d_diag = nc.dram_tensor("diag", [P, P], f32, kind="ExternalInput")
    d_out = nc.dram_tensor("out", [T, D], f32, kind="ExternalOutput")

    with tile.TileContext(nc) as tc, ExitStack() as ctx:
        const = ctx.enter_context(tc.tile_pool(name="const", bufs=1))
        small = ctx.enter_context(tc.tile_pool(name="small", bufs=2))
        psum_mm = ctx.enter_context(tc.tile_pool(name="psum_mm", bufs=5, space="PSUM"))
        psum_tr = ctx.enter_context(tc.tile_pool(name="psum_tr", bufs=1, space="PSUM"))
        psum_rs = ctx.enter_context(tc.tile_pool(name="psum_rs", bufs=2, space="PSUM"))

        # ---- constants / small inputs --------------------------------
        ident_f = const.tile([P, P], f32)
        make_identity(nc, ident_f)
        ones_sum = const.tile([P, 2, P], fp8)
        nc.vector.memset(ones_sum[:], 1.0 / USC)
        ones_row = const.tile([1, P], bf16)
        nc.vector.memset(ones_row[:], 1.0)
        eps_t = const.tile([P, 1], f32)
        nc.vector.memset(eps_t[:], LN_EPS)
        diag_sb = const.tile([P, P], f32)
        nc.gpsimd.dma_start(out=diag_sb[:], in_=d_diag.ap())
        tpad_sb = const.tile([P, NS], f32)
        nc.gpsimd.dma_start(out=tpad_sb[:], in_=d_tpad.ap())
        spad_sb = const.tile([P, NS], f32)
        nc.gpsimd.dma_start(out=spad_sb[:], in_=d_spad.ap())
        cq_sb = const.tile([P, ND, H], fp8)
        nc.gpsimd.dma_start(out=cq_sb[:], in_=d_cq8.ap())
        cm_sb = const.tile([P, ND, H], fp8)
        nc.gpsimd.dma_start(out=cm_sb[:], in_=d_cm8.ap())
        b1_sb = const.tile([P, NF], f32)
        nc.gpsimd.dma_start(out=b1_sb[:], in_=d_b1.ap())
        bo_sb = const.tile([1, D], bf16)
        nc.gpsimd.dma_start(out=bo_sb[:], in_=d_bo.ap())
        bom_sb = const.tile([1, D], bf16)
        nc.gpsimd.dma_start(out=bom_sb[:], in_=d_bom.ap())
        b2_sb = const.tile([1, D], bf16)
        nc.gpsimd.dma_start(out=b2_sb[:], in_=d_b2.ap())

        # ---- pools with phase-scoped lifetimes (LIFO close order) ----
        es_x2 = ExitStack()     # x2/x2T: FFN phase
        x2_pool = es_x2.enter_context(tc.tile_pool(name="x2p", bufs=1))
        es_attn = ExitStack()   # expT + uTall: self + cross attention
        attn_pool = es_attn.enter_context(tc.tile_pool(name="attn", bufs=1))
        es_x1 = ExitStack()     # x1/x1T: cross phase
        x1_pool = es_x1.enter_context(tc.tile_pool(name="x1p", bufs=1))
        es_x32 = ExitStack()    # x (f32/fp8/T), self radd: self phase
        x32_pool = es_x32.enter_context(tc.tile_pool(name="x32p", bufs=1))

        expT = attn_pool.tile([P, NS, T], fp8, tag="expT")
        nc.gpsimd.memset(expT[:], 0.0)
        uTall = attn_pool.tile([P, H, ND, T], fp8, tag="uTall")
        # xT8 feeds the very first PE work (radd + qMT) - DMA it first;
        # x32 is only needed at LN1, keep it off the sync queue entirely.
        xT8 = x32_pool.tile([P, ND, T], fp8, tag="xT8")
        nc.sync.dma_start(
            out=xT8[:], in_=d_xT8.ap().rearrange("(eb p) t -> p eb t", p=P))
        x8_sb = x32_pool.tile([P, NT, D], fp8, tag="x8")
        nc.sync.dma_start(
            out=x8_sb[:], in_=d_x8.ap().rearrange("(tb p) d -> p tb d", p=P))
        x32_sb = x32_pool.tile([P, NT, D], f32, tag="x32")
        nc.gpsimd.dma_start(
            out=x32_sb[:], in_=d_x32.ap().rearrange("(tb p) d -> p tb d", p=P))

        def transpose_to(src_ap, dstT, tb):
            for dt in range(ND):
                tr_ps = psum_tr.tile([P, P], f32, tag="tr")
                nc.tensor.transpose(
                    tr_ps[:], src_ap[:, dt * P:(dt + 1) * P], ident_f[:])
                nc.vector.tensor_copy(dstT[:, dt, tb * P:(tb + 1) * P], tr_ps[:])

        def layernorm(src_ap, scale, resid_ap, dst_ap):
            res = small.tile([P, D], f32, tag="ln_res")
            if scale is None:
                nc.vector.tensor_tensor(
                    out=res[:], in0=src_ap, in1=resid_ap, op=OP.add)
            else:
                nc.vector.scalar_tensor_tensor(
                    out=res[:], in0=src_ap, scalar=scale, in1=resid_ap,
                    op0=OP.mult, op1=OP.add)
            stats = small.tile([P, 6], f32, tag="ln_stats")
            nc.vector.bn_stats(stats[:], res[:])
            mv = small.tile([P, 2], f32, tag="ln_mv")
            nc.vector.bn_aggr(mv[:], stats[:])
            std = small.tile([P, 1], f32, tag="ln_std")
            nc.scalar.activation(std[:], mv[:, 1:2], AF.Sqrt, bias=eps_t[:])
            istd = small.tile([P, 1], f32, tag="ln_istd")
            nc.vector.reciprocal(istd[:], std[:])
            nc.vector.tensor_scalar(
                out=dst_ap, in0=res[:], scalar1=mv[:, 0:1], scalar2=istd[:],
                op0=OP.subtract, op1=OP.mult)

        def build_radd(lhsT_sb, c_sb, pad_sb, radd):
            """radd[:, j, h] = pad[:, j] + RINV * sum_d lhsT[d, s] c[d, h]."""
            for j in range(NS):
                r_ps = psum_mm.tile([P, 512], f32, tag="mm")
                for kp in range(ND // 2):
                    nc.tensor.matmul(
                        r_ps[:, 0:H], lhsT=lhsT_sb[:, 2 * kp:2 * kp + 2,
                                                   j * P:(j + 1) * P],
                        rhs=c_sb[:, 2 * kp:2 * kp + 2, :], perf_mode=DR,
                        start=(kp == 0), stop=(kp == ND // 2 - 1))
                nc.vector.tensor_scalar(
                    out=radd[:, j, :], in0=r_ps[:, 0:H], scalar1=RINV,
                    scalar2=pad_sb[:, j:j + 1], op0=OP.mult, op1=OP.add)

        def proj_qT(w_t, src_T8, dstT8):
            """dstT8[e, t] = sum_d w[d, e] src[d, t]   (both fp8, DoubleRow)."""
            for eb in range(ND):
                for c in range(NC2):
                    ps = psum_mm.tile([P, 512], f32, tag="mm")
                    for kp in range(ND // 2):
                        nc.tensor.matmul(
                            ps[:], lhsT=w_t[:, 2 * kp:2 * kp + 2, eb, :],
                            rhs=src_T8[:, 2 * kp:2 * kp + 2,
                                       c * 512:(c + 1) * 512],
                            perf_mode=DR, start=(kp == 0),
                            stop=(kp == ND // 2 - 1))
                    nc.scalar.activation(
                        dstT8[:, eb, c * 512:(c + 1) * 512], ps[:], AF.Copy)

        def rowsum_chunk(c, jmax, recip_bc):
            """Rowsum with a 128-wide all-ones stationary: every PSUM
            partition receives the sum, so the reciprocal runs full-width
            and no partition broadcast is needed."""
            sl = slice(c * 512, (c + 1) * 512)
            rs_ps = psum_rs.tile([P, 512], f32, tag="rs")
            for kp in range(jmax // 2):
                nc.tensor.matmul(
                    rs_ps[:], lhsT=ones_sum[:],
                    rhs=expT[:, 2 * kp:2 * kp + 2, sl], perf_mode=DR,
                    start=(kp == 0), stop=(kp == jmax // 2 - 1))
            nc.vector.reciprocal(recip_bc[:, sl], rs_ps[:])

        def u_chunk(h, c, jmax, v_sb, recip_bc):
            sl = slice(c * 512, (c + 1) * 512)
            for eb in range(ND):
                ps = psum_mm.tile([P, 512], f32, tag="mm")
                for kp in range(jmax // 2):
                    nc.tensor.matmul(
                        ps[:], lhsT=v_sb[:, 2 * kp:2 * kp + 2,
                                         eb * P:(eb + 1) * P],
                        rhs=expT[:, 2 * kp:2 * kp + 2, sl], perf_mode=DR,
                        start=(kp == 0), stop=(kp == jmax // 2 - 1))
                nc.vector.tensor_tensor(
                    out=uTall[:, h, eb, sl], in0=ps[:], in1=recip_bc[:, sl],
                    op=OP.mult)

        def oproj_ln(wall, brow_sb, resid_sb, dst_sb, dstT):
            """Per t-block: accumulate all heads' uT @ W in one PSUM group,
            add the bias row, then LN(psum*OSC + resid) -> dst (+transpose)."""
            prev = -1
            for tb in range(NT):
                ps = psum_mm.tile([P, 512], f32, tag="mm")
                for h in range(H):
                    for kp in range(ND // 2):
                        nc.tensor.matmul(
                            ps[:], lhsT=uTall[:, h, 2 * kp:2 * kp + 2,
                                              tb * P:(tb + 1) * P],
                            rhs=wall[:, h, 2 * kp:2 * kp + 2, :],
                            perf_mode=DR, start=(h == 0 and kp == 0),
                            stop=False)
                nc.tensor.matmul(
                    ps[:], lhsT=ones_row[:, 0:P], rhs=brow_sb[:],
                    start=False, stop=True)
                layernorm(ps[:], OSC, resid_sb[:, tb, :], dst_sb[:, tb, :])
                if prev >= 0:
                    transpose_to(dst_sb[:, prev, :], dstT, prev)
                prev = tb
            transpose_to(dst_sb[:, prev, :], dstT, prev)

        # ============ phase 1: self attention =========================
        radd_s = x32_pool.tile([P, NS, H], f32, tag="radd_s")
        build_radd(xT8, cq_sb, tpad_sb, radd_s)

        with tc.tile_pool(name="gallp", bufs=1) as gall_pool, \
             tc.tile_pool(name="qkv", bufs=2) as qkv_pool, \
             tc.tile_pool(name="wstream", bufs=2) as wstream, \
             tc.tile_pool(name="rbc", bufs=2) as rbc_pool:
            gall = gall_pool.tile([P, H, ND, D], fp8)
            nc.gpsimd.dma_start(out=gall[:], in_=d_g8.ap().rearrange(
                "(h kt p) e -> p h kt e", p=P, h=H))

            def load_m(h):
                m_t = wstream.tile([P, ND, ND, P], fp8, tag="m")
                nc.sync.dma_start(out=m_t[:], in_=d_m8.ap()[h].rearrange(
                    "(kt p) (eb e) -> p kt eb e", p=P, e=P))
                return m_t

            def att_block(j, c, qT, radd, h):
                lo = max(j * P, c * 512)
                w = (c + 1) * 512 - lo
                ps = psum_mm.tile([P, 512], f32, tag="mm")
                for kp in range(ND // 2):
                    nc.tensor.matmul(
                        ps[:, :w],
                        lhsT=xT8[:, 2 * kp:2 * kp + 2, j * P:(j + 1) * P],
                        rhs=qT[:, 2 * kp:2 * kp + 2, lo:(c + 1) * 512],
                        perf_mode=DR, start=(kp == 0),
                        stop=(kp == ND // 2 - 1))
                if lo == j * P:
                    nc.vector.tensor_tensor(
                        out=ps[:, 0:P], in0=ps[:, 0:P], in1=diag_sb[:],
                        op=OP.add)
                nc.scalar.activation(
                    expT[:, j, lo:(c + 1) * 512], ps[:, :w], AF.Exp,
                    bias=radd[:, j, h:h + 1], scale=SCALE / WSC_M)

            m_t = load_m(0)
            qMT = qkv_pool.tile([P, ND, T], fp8, tag="qMT")
            proj_qT(m_t, xT8, qMT)
            for h in range(H):
                if h + 1 < H:
                    m_t = load_m(h + 1)
                recip_bc = rbc_pool.tile([P, T], f32, tag="recip_bc")
                for j in range(4):
                    att_block(j, 0, qMT, radd_s, h)
                    att_block(j, 1, qMT, radd_s, h)
                rowsum_chunk(0, 4, recip_bc)
                u_chunk(h, 0, 4, x8_sb, recip_bc)
                for j in range(4, NS):
                    att_block(j, 1, qMT, radd_s, h)
                if h + 1 < H:
                    qMT_next = qkv_pool.tile([P, ND, T], fp8, tag="qMT")
                    proj_qT(m_t, xT8, qMT_next)
                rowsum_chunk(1, NS, recip_bc)
                u_chunk(h, 1, NS, x8_sb, recip_bc)
                if h + 1 < H:
                    qMT = qMT_next

            x1_sb = x1_pool.tile([P, NT, D], f32, tag="x1")
            x1T8 = x1_pool.tile([P, ND, T], fp8, tag="x1T")
            oproj_ln(gall, bo_sb, x32_sb, x1_sb, x1T8)
        es_x32.close()

        # ============ phase 2: cross attention ========================
        with tc.tile_pool(name="mem", bufs=1) as mem_pool, \
             tc.tile_pool(name="qkv2", bufs=2) as qkv2_pool, \
             tc.tile_pool(name="wstream2", bufs=2) as wstream2, \
             tc.tile_pool(name="rbc2", bufs=2) as rbc2_pool, \
             tc.tile_pool(name="ffn", bufs=1) as ffn_pool:
            # FFN weights DMA'd now so they are resident long before phase 3
            w1_t = ffn_pool.tile([P, ND, NF, P], bf16, tag="w1")
            nc.sync.dma_start(out=w1_t[:], in_=d_w1.ap().rearrange(
                "(kt p) (fb f) -> p kt fb f", p=P, f=P))
            w2_t = ffn_pool.tile([P, NF, D], bf16, tag="w2")
            nc.sync.dma_start(out=w2_t[:], in_=d_w2.ap().rearrange(
                "(kt p) d -> p kt d", p=P))
            memkT8 = mem_pool.tile([P, ND, S], fp8, tag="memkT")
            nc.gpsimd.dma_start(out=memkT8[:], in_=d_memkT8.ap().rearrange(
                "(eb p) s -> p eb s", p=P))
            memv8 = mem_pool.tile([P, NS, D], fp8, tag="memv")
            nc.gpsimd.dma_start(out=memv8[:], in_=d_memv8.ap().rearrange(
                "(st p) e -> p st e", p=P))
            womall = mem_pool.tile([P, H, ND, D], fp8, tag="womall")
            nc.gpsimd.dma_start(out=womall[:], in_=d_wom8.ap().rearrange(
                "(h kt p) e -> p h kt e", p=P, h=H))
            radd_m = mem_pool.tile([P, NS, H], f32, tag="radd_m")
            build_radd(memkT8, cm_sb, spad_sb, radd_m)

            def load_wqm(h):
                wqm_t = wstream2.tile([P, ND, ND, P], fp8, tag="wqm")
                nc.sync.dma_start(out=wqm_t[:], in_=d_wqm8.ap()[h].rearrange(
                    "(kt p) (eb e) -> p kt eb e", p=P, e=P))
                return wqm_t

            def attm_block(j, c, qT, h):
                ps = psum_mm.tile([P, 512], f32, tag="mm")
                for kp in range(ND // 2):
                    nc.tensor.matmul(
                        ps[:],
                        lhsT=memkT8[:, 2 * kp:2 * kp + 2, j * P:(j + 1) * P],
                        rhs=qT[:, 2 * kp:2 * kp + 2, c * 512:(c + 1) * 512],
                        perf_mode=DR, start=(kp == 0),
                        stop=(kp == ND // 2 - 1))
                nc.scalar.activation(
                    expT[:, j, c * 512:(c + 1) * 512], ps[:], AF.Exp,
                    bias=radd_m[:, j, h:h + 1], scale=SCALE / WSC_QM)

            wqm_t = load_wqm(0)
            qmT = qkv2_pool.tile([P, ND, T], fp8, tag="qmT")
            proj_qT(wqm_t, x1T8, qmT)
            for h in range(H):
                if h + 1 < H:
                    wqm_t = load_wqm(h + 1)
                recip_bc = rbc2_pool.tile([P, T], f32, tag="recip_bc")
                for c in range(NC2):
                    for j in range(NS):
                        attm_block(j, c, qmT, h)
                rowsum_chunk(0, NS, recip_bc)
                u_chunk(h, 0, NS, memv8, recip_bc)
                if h + 1 < H:
                    qmT_next = qkv2_pool.tile([P, ND, T], fp8, tag="qmT")
                    proj_qT(wqm_t, x1T8, qmT_next)
                rowsum_chunk(1, NS, recip_bc)
                u_chunk(h, 1, NS, memv8, recip_bc)
                if h + 1 < H:
                    qmT = qmT_next

            x2_sb = x2_pool.tile([P, NT, D], f32, tag="x2")
            x2T = x2_pool.tile([P, ND, T], bf16, tag="x2T")
            oproj_ln(womall, bom_sb, x1_sb, x2_sb, x2T)

            # ============ phase 3: FFN + LN3 (bf16) ===================
            f1T = ffn_pool.tile([P, NF, T], bf16, tag="f1T")

            def f1_chunk(c):
                for fb in range(NF):
                    f_ps = psum_mm.tile([P, 512], f32, tag="mm")
                    for kt in range(ND):
                        nc.tensor.matmul(
                            f_ps[:], lhsT=w1_t[:, kt, fb, :],
                            rhs=x2T[:, kt, c * 512:(c + 1) * 512],
                            start=(kt == 0), stop=(kt == ND - 1))
                    nc.scalar.activation(
                        f1T[:, fb, c * 512:(c + 1) * 512], f_ps[:], AF.Relu,
                        bias=b1_sb[:, fb:fb + 1])

            f1_chunk(0)
            f1_chunk(1)
            for tb in range(NT):
                o_ps = psum_mm.tile([P, 512], f32, tag="mm")
                for kt in range(NF):
                    nc.tensor.matmul(
                        o_ps[:], lhsT=f1T[:, kt, tb * P:(tb + 1) * P],
                        rhs=w2_t[:, kt, :], start=(kt == 0), stop=False)
                nc.tensor.matmul(
                    o_ps[:], lhsT=ones_row[:, 0:P], rhs=b2_sb[:],
                    start=False, stop=True)
                out_sb = small.tile([P, D], f32, tag="out_sb")
                layernorm(o_ps[:], None, x2_sb[:, tb, :], out_sb[:])
                nc.sync.dma_start(
                    out=d_out.ap().rearrange("(tb p) d -> p tb d", p=P)[:, tb, :],
                    in_=out_sb[:])
        es_x1.close()
        es_attn.close()
        es_x2.close()

    nc.compile()
    _CACHE["nc"] = nc
    return nc


def make_in_maps(inputs):
    import ml_dtypes

    bf = ml_dtypes.bfloat16
    f8 = ml_dtypes.float8_e4m3
    f32 = np.float32

    wq = np.asarray(inputs["Wq_self"], f32)
    wk = np.asarray(inputs["Wk_self"], f32)
    wv = np.asarray(inputs["Wv_self"], f32)
    wo = np.asarray(inputs["Wo_self"], f32)
    bq = np.asarray(inputs["bq_self"], f32)
    bv = np.asarray(inputs["bv_self"], f32)
    wqm = np.asarray(inputs["Wq_mem"], f32)
    wom = np.asarray(inputs["Wo_mem"], f32)
    bqm = np.asarray(inputs["bq_mem"], f32)

    # fold Q/K into M = Wq Wk^T, V into G = Wv Wo_h
    m = np.matmul(wq, wk.transpose(0, 2, 1))                 # [H, D, D]
    g = np.matmul(wv, wo.reshape(H, D, D))                   # [H, D, D]
    # per-key score bias: x @ (Wk bq); the Wq bk term is softmax-invariant
    cq = np.einsum('hde,he->dh', wk, bq) * (RSC * SCALE)     # [D, H]
    cm = bqm.T * (RSC * SCALE)                               # [D, H]

    bo_row = np.asarray(inputs["bo_self"], f32).copy()
    for h in range(H):
        bo_row += bv[h] @ wo[h * D:(h + 1) * D]

    def colh(a):  # [D, H] -> [P, ND, H]
        return np.ascontiguousarray(a.reshape(ND, P, H).transpose(1, 0, 2))

    shared = {
        "m8": (m * WSC_M).astype(f8),
        "g8": (g * WSC_O).reshape(H * D, D).astype(f8),
        "cq8": colh(cq).astype(f8),
        "wqm8": (wqm * WSC_QM).astype(f8),
        "wom8": (wom * WSC_O).astype(f8),
        "cm8": colh(cm).astype(f8),
        "w1": np.asarray(inputs["W1"], f32).astype(bf),
        "w2": np.asarray(inputs["W2"], f32).astype(bf),
        "b1_c": np.ascontiguousarray(
            np.asarray(inputs["b1"], f32).reshape(NF, P).T).astype(f32),
        "bo_row": (bo_row * (WSC_O * USC)).reshape(1, D).astype(bf),
        "bom_row": (np.asarray(inputs["bo_mem"], f32)
                    * (WSC_O * USC)).reshape(1, D).astype(bf),
        "b2_row": np.asarray(inputs["b2"], f32).reshape(1, D).astype(bf),
        # attT is [s, t]: transpose the causal diagonal block
        "diag": np.ascontiguousarray(
            np.asarray(inputs["tgt_subsq_mask"], f32)[:P, :P].T),
    }
    in_maps = []
    for b in range(B):
        xb = np.asarray(inputs["x"], f32)[b]
        mk = np.asarray(inputs["mem_keys"], f32)[b]
        mvv = np.asarray(inputs["mem_values"], f32)[b]
        mm = dict(shared)
        mm["x32"] = np.ascontiguousarray(xb)
        mm["x8"] = xb.astype(f8)
        mm["xT8"] = np.ascontiguousarray(xb.T).astype(f8)
        mm["memkT8"] = np.ascontiguousarray(mk.T).astype(f8)
        mm["memv8"] = mvv.astype(f8)
        mm["tpad"] = np.ascontiguousarray(
            np.asarray(inputs["tgt_padding_mask"], f32)[b, :, 0].reshape(NS, P).T)
        mm["spad"] = np.ascontiguousarray(
            np.asarray(inputs["src_padding_mask"], f32)[b, :, 0].reshape(NS, P).T)
        in_maps.append(mm)
    return in_maps


def kernel(**inputs):
    from concourse.bass_utils import run_bass_kernel_spmd

    nc = _build()
    in_maps = make_in_maps(inputs)
    res = run_bass_kernel_spmd(nc, in_maps, list(range(B)))
    out = np.stack([np.asarray(res.results[i]["out"]) for i in range(B)])
    return out.astype(np.float32)
